# revision 7
# baseline (speedup 1.0000x reference)
"""ChannelAttentionV2 Trainium2 kernel (8 NeuronCores, data-parallel over batch).

Math (per batch b, per head h; N=4096 tokens, C=768, D=96):
  q = x @ wq.T + bq ; k = x @ wk.T + bk ; v = x @ wv.T + bv
  L = (q * N^-0.5).T @ k            [D, D] logits
  A = softmax(L, axis=-1)
  out_h = (A @ v.T).T               [N, D]
  final = concat_h(out_h) @ w_proj.T + b_proj

Kernel reformulation (per core, 2 batches), all matmuls in fp16
(psum accumulation is fp32; softmax in fp32):
  G  = x.T @ x  (upper blocks only, lower restored by PE-transpose symmetry)
  xT = PE-transpose of x tiles, kept in SBUF (feeds phase C + column sums)
  cs = free-dim reduce of xT (DVE)  ->  csT columns directly
  L  = s*(wq G wk.T + bq (x) u + sq (x) bk), u = sk + N*bk, sq/sk = cs @ wq/wk.T
  A  = softmax(L)
  Mcat_h[e,co] = sum_d A[d,e] w_proj[co, h*96+d]
  W2 = wv.T @ Mcat ; bias_row = bv @ Mcat + b_proj
  final = xT.T @ W2 + bias_row   (bias via rank-1 matmul or fused DVE/Pool add)
"""

import numpy as np

import concourse.bass as bass
import concourse.mybir as mybir
import concourse.tile as tile
from concourse import bacc
from concourse import bass_utils
from concourse.masks import make_identity

F32 = mybir.dt.float32
F16 = mybir.dt.float16

NCORES = 8
B_TOT = 16
BLOC = B_TOT // NCORES  # 2 batches per core
N = 4096
C = 768
H = 8
D = 96
CK = C // 128  # 6 chunks of channels
NT = N // 128  # 32 token tiles per batch
SCALE = float(N) ** -0.5  # 1/64

PHASE_MARKS = []  # (instruction_id_watermark, label) for profiling


def _mark(nc, label):
    try:
        name = nc.get_next_instruction_name()  # consumes one id
        PHASE_MARKS.append((int(name.split("-")[1]), label))
    except Exception:
        pass


def _build_kernel_body(nc, tc, aps):
    x = aps["x"]
    w_qkv = aps["w_qkv"]
    b_qkv = aps["b_qkv"]
    w_proj = aps["w_proj"]
    b_proj = aps["b_proj"]
    out = aps["out"]

    import contextlib

    ctx = contextlib.ExitStack()
    with ctx:
        singles = ctx.enter_context(tc.tile_pool(name="singles", bufs=1))
        xpool = ctx.enter_context(tc.tile_pool(name="xpool", bufs=8))
        wnpool = ctx.enter_context(tc.tile_pool(name="wnpool", bufs=2))
        wt_pool = ctx.enter_context(tc.tile_pool(name="wt", bufs=1))
        g_pool = ctx.enter_context(tc.tile_pool(name="gpool", bufs=1))
        a1_pool = ctx.enter_context(tc.tile_pool(name="a1", bufs=1))
        mcat_pool = ctx.enter_context(tc.tile_pool(name="mcat", bufs=1))
        w2_pool = ctx.enter_context(tc.tile_pool(name="w2", bufs=1))
        xt_pool = ctx.enter_context(tc.tile_pool(name="xt", bufs=2))
        outpool = ctx.enter_context(tc.tile_pool(name="outp", bufs=2))
        smalls = ctx.enter_context(tc.tile_pool(name="smalls", bufs=1))
        ps = ctx.enter_context(tc.tile_pool(name="ps", bufs=1, space="PSUM"))

        _psum_ctr = [0]

        def psum(shape, tag, bufs, dtype=F32):
            _psum_ctr[0] += 1
            return ps.tile(
                shape, dtype, tag=tag, bufs=bufs, name=f"ps_{tag}_{_psum_ctr[0]}"
            )

        # round-robin copy engine (gpsimd cannot access PSUM)
        _rr = [0]

        def eng_copy(dst, src, pattern="da"):
            e = pattern[_rr[0] % len(pattern)]
            _rr[0] += 1
            if e == "d":
                nc.vector.tensor_copy(dst, src)
            else:
                nc.scalar.copy(dst, src)

        # --------- tiny constants ---------
        ident16 = singles.tile([128, 128], F16)
        make_identity(nc, ident16)
        ones_row16 = singles.tile([1, 128], F16)
        nc.vector.memset(ones_row16, 1.0)

        # persistent weight tiles (all fp16)
        wt_qk = [
            wt_pool.tile([128, 2 * C], F16, tag=f"wtqk{j}", name=f"wtqk{j}")
            for j in range(CK)
        ]
        wpT = [
            wt_pool.tile([128, C], F16, tag=f"wpT{h}", name=f"wpT{h}")
            for h in range(H)
        ]
        wv_t = [
            wt_pool.tile([128, C], F16, tag=f"wv{h}", name=f"wv{h}")
            for h in range(H)
        ]
        bq_r = singles.tile([1, C], F16)
        bk_r = singles.tile([1, C], F16)
        bk_f = singles.tile([1, C], F32)
        bp_f = singles.tile([1, C], F32)
        bv_col = [
            singles.tile([128, 1], F16, tag=f"bv{h}", name=f"bv{h}") for h in range(H)
        ]

        def emit_setup():
            # wt_qk[j][c in chunk j, 1536] = w_qkv[0:1536, :].T  (fp16)
            for i in range(2 * CK):  # 12 row-chunks of w_qkv[0:1536]
                nat = wnpool.tile([128, C], F16, tag="wn", name="wnat")
                nc.gpsimd.dma_start(nat, w_qkv[i * 128 : (i + 1) * 128, :])
                for jg in range(2):
                    pt = psum([128, 384], "tp", 2, F16)
                    for j3 in range(3):
                        j = jg * 3 + j3
                        nc.tensor.transpose(
                            pt[:, j3 * 128 : (j3 + 1) * 128],
                            nat[:, j * 128 : (j + 1) * 128],
                            ident16,
                        )
                    for j3 in range(3):
                        j = jg * 3 + j3
                        eng_copy(
                            wt_qk[j][:, i * 128 : (i + 1) * 128],
                            pt[:, j3 * 128 : (j3 + 1) * 128],
                        )

            # wpT[h][d(96), co=768] = w_proj[:, h*96+d].T  (fp16)
            for i in range(CK):  # co-chunks of w_proj
                nat = wnpool.tile([128, C], F16, tag="wn", name="wpnat")
                nc.gpsimd.dma_start(nat, w_proj[i * 128 : (i + 1) * 128, :])
                for hg in range(4):  # head groups of 2
                    pt = psum([128, 384], "tp", 2, F16)
                    for h2 in range(2):
                        h = hg * 2 + h2
                        nc.tensor.transpose(
                            pt[0:D, h2 * 128 : (h2 + 1) * 128],
                            nat[:, h * D : (h + 1) * D],
                            ident16,
                        )
                    for h2 in range(2):
                        h = hg * 2 + h2
                        eng_copy(
                            wpT[h][0:D, i * 128 : (i + 1) * 128],
                            pt[0:D, h2 * 128 : (h2 + 1) * 128],
                        )

            # wv_t[h][e(96), ci=768] = w_qkv[2C + h*96 + e, :]  (fp16)
            for h in range(H):
                nc.gpsimd.dma_start(
                    wv_t[h][0:D, :], w_qkv[2 * C + h * D : 2 * C + (h + 1) * D, :]
                )

            # bias rows
            nc.gpsimd.dma_start(bq_r, b_qkv[None, 0:C])
            nc.gpsimd.dma_start(bk_r, b_qkv[None, C : 2 * C])
            nc.sync.dma_start(bk_f, b_qkv[None, C : 2 * C])
            for h in range(H):
                nc.gpsimd.dma_start(
                    bv_col[h][0:D, :],
                    b_qkv[2 * C + h * D : 2 * C + (h + 1) * D, None],
                )
            nc.sync.dma_start(bp_f, b_proj[None, :])

        # ---------------- per batch ----------------
        for b in range(BLOC):
            _mark(nc, "phaseA")
            # x tile loads: 8 cast-DMAs of 4 token-tiles each (f32 -> fp16)
            xt4 = []
            for i in range(NT // 4):
                xt = xpool.tile([128, 4, C], F16, tag="xt", name="xt")
                r0 = i * 512
                nc.gpsimd.dma_start(
                    xt,
                    x[b, r0 : r0 + 512, :].rearrange("(t p) c -> p t c", p=128),
                )
                xt4.append(xt)

            def xts(kk):
                return xt4[kk >> 2][:, kk & 3, :]

            if b == 0:
                _mark(nc, "setup")
                emit_setup()

            _mark(nc, "xT")
            # PE-transpose x -> xT (fp16), kept in SBUF for phase C
            # layout: xTg[g][c-part(128), tok-tile(32), j(3), tok(128)]
            # where channel chunk k = 3*g + j
            xTg = [
                xt_pool.tile([128, NT, 3, 128], F16, tag="xT", name=f"xT{b}_{g}")
                for g in range(2)
            ]
            for kk in range(NT):
                for g in range(2):
                    pt = psum([128, 384], "tp", 2, F16)
                    for j in range(3):
                        nc.tensor.transpose(
                            pt[:, j * 128 : (j + 1) * 128],
                            xts(kk)[:, (3 * g + j) * 128 : (3 * g + j + 1) * 128],
                            ident16,
                        )
                    eng_copy(xTg[g][:, kk, :, :], pt, "dda")

            # column sums via DVE free-dim reduce on xT -> csT columns
            csT_f = smalls.tile([128, CK], F32, tag="csT", name="csT")
            for k in range(CK):
                nc.vector.tensor_reduce(
                    csT_f[:, k : k + 1],
                    xTg[k // 3][:, :, k % 3, :],
                    axis=mybir.AxisListType.XY,
                    op=mybir.AluOpType.add,
                )
            csT = smalls.tile([128, CK], F16, tag="csT16", name="csT16")
            nc.vector.tensor_copy(csT, csT_f)

            _mark(nc, "gram")
            # G = x.T @ x: upper 384-wide blocks, 32-matmul psum accumulation
            g_t = [
                g_pool.tile([128, C], F16, tag=f"g{m}", name=f"g{m}")
                for m in range(CK)
            ]
            for m in range(CK):
                for nh in range(2):
                    if m * 128 >= (nh + 1) * 384:
                        continue  # below-diagonal half: restored by symmetry
                    pt = psum([128, 384], "big", 2)
                    for kk in range(NT):
                        nc.tensor.matmul(
                            pt,
                            xts(kk)[:, m * 128 : (m + 1) * 128],
                            xts(kk)[:, nh * 384 : (nh + 1) * 384],
                            start=(kk == 0),
                            stop=(kk == NT - 1),
                        )
                    nc.vector.tensor_copy(g_t[m][:, nh * 384 : (nh + 1) * 384], pt)

            _mark(nc, "gmirror")
            # mirror below-diagonal blocks of G by transposing the upper ones
            for mi in range(3, CK):
                pt = psum([128, 384], "mc", 2, F16)
                for mj in range(3):
                    nc.tensor.transpose(
                        pt[:, mj * 128 : (mj + 1) * 128],
                        g_t[mj][:, mi * 128 : (mi + 1) * 128],
                        ident16,
                    )
                nc.vector.tensor_copy(g_t[mi][:, 0:384], pt)

            _mark(nc, "sall")
            # s = cs @ [wq|wk].T : sq (fp16 rank-1 lhsT) and sk (f32, for u)
            sq_r = smalls.tile([1, C], F16, tag="sqr", name="sqr")
            sk_f = smalls.tile([1, C], F32, tag="skf", name="skf")
            for seg in range(4):
                pt = psum([128, 384], "mc", 2)[0:1, :]
                for j in range(CK):
                    nc.tensor.matmul(
                        pt,
                        csT[:, j : j + 1],
                        wt_qk[j][:, seg * 384 : (seg + 1) * 384],
                        start=(j == 0),
                        stop=(j == CK - 1),
                    )
                if seg < 2:
                    nc.vector.tensor_copy(sq_r[:, seg * 384 : (seg + 1) * 384], pt)
                else:
                    nc.vector.tensor_copy(
                        sk_f[:, (seg - 2) * 384 : (seg - 1) * 384], pt
                    )

            # u = sk + N * bk   (fp16 row)
            u_f = smalls.tile([1, C], F32, tag="uf", name="uf")
            u_r = smalls.tile([1, C], F16, tag="ur", name="ur")
            nc.vector.tensor_scalar(u_f, bk_f, float(N), None, op0=mybir.AluOpType.mult)
            nc.vector.tensor_add(u_f, u_f, sk_f)
            nc.vector.tensor_copy(u_r, u_f)

            _mark(nc, "A1T")
            # A1T[c', d_all] = sum_c G[c, c'] * wq[d_all, c]
            a1t = [
                a1_pool.tile([128, C], F16, tag=f"a1t{m}", name=f"a1t{m}")
                for m in range(CK)
            ]
            for m in range(CK):
                for nh in range(2):
                    pt = psum([128, 384], "big", 2)
                    for k in range(CK):
                        nc.tensor.matmul(
                            pt,
                            g_t[k][:, m * 128 : (m + 1) * 128],
                            wt_qk[k][:, nh * 384 : (nh + 1) * 384],
                            start=(k == 0),
                            stop=(k == CK - 1),
                        )
                    eng_copy(a1t[m][:, nh * 384 : (nh + 1) * 384], pt, "da")

            _mark(nc, "heads")
            # per-head logits + softmax + Mcat
            mcat = [
                mcat_pool.tile([128, C], F16, tag=f"mcat{h}", name=f"mcat{h}")
                for h in range(H)
            ]
            for h in range(H):
                lp = psum([128, 96], "attn", 2)[0:D, :]
                for k in range(CK):
                    nc.tensor.matmul(
                        lp,
                        a1t[k][:, h * D : (h + 1) * D],
                        wt_qk[k][:, C + h * D : C + (h + 1) * D],
                        start=(k == 0),
                        stop=False,
                    )
                # rank-1 bias terms: bq (x) u  and  sq (x) bk
                nc.tensor.matmul(
                    lp,
                    bq_r[:, h * D : (h + 1) * D],
                    u_r[:, h * D : (h + 1) * D],
                    start=False,
                    stop=False,
                )
                nc.tensor.matmul(
                    lp,
                    sq_r[:, h * D : (h + 1) * D],
                    bk_r[:, h * D : (h + 1) * D],
                    start=False,
                    stop=True,
                )
                # softmax over free dim, scale folded into the exp
                negm = smalls.tile([128, 1], F32, tag="negm", name="negm")[0:D, :]
                nc.vector.tensor_reduce(
                    negm, lp, axis=mybir.AxisListType.X, op=mybir.AluOpType.max,
                    negate=True,
                )
                negm_s = smalls.tile([128, 1], F32, tag="negms", name="negms")[0:D, :]
                nc.vector.tensor_scalar_mul(negm_s, negm, SCALE)
                p_t = smalls.tile([128, 96], F32, tag="pt", name="pt")[0:D, :]
                ssum = smalls.tile([128, 1], F32, tag="ssum", name="ssum")[0:D, :]
                nc.scalar.activation(
                    p_t, lp, mybir.ActivationFunctionType.Exp,
                    bias=negm_s, scale=SCALE, accum_out=ssum,
                )
                rinv = smalls.tile([128, 1], F32, tag="rinv", name="rinv")[0:D, :]
                nc.vector.reciprocal(rinv, ssum)
                attn16 = smalls.tile([128, 96], F16, tag="attn16", name="attn16")[
                    0:D, :
                ]
                nc.vector.tensor_scalar_mul(attn16, p_t, rinv)
                # Mcat_h[e, co] = sum_d attn[d, e] * wpT[h][d, co]
                for nh in range(2):
                    pt = psum([128, 384], "mc", 2)[0:D, :]
                    nc.tensor.matmul(
                        pt, attn16, wpT[h][0:D, nh * 384 : (nh + 1) * 384],
                        start=True, stop=True,
                    )
                    eng_copy(mcat[h][0:D, nh * 384 : (nh + 1) * 384], pt, "da")

            _mark(nc, "W2")
            # W2 = wv.T-contract @ Mcat   [ci, co] (fp16)
            # reuses the G tiles' storage: g is dead after A1T
            w2 = [
                g_pool.tile([128, C], F16, tag=f"g{m}", name=f"w2{m}")
                for m in range(CK)
            ]
            for m in range(CK):
                for nh in range(2):
                    pt = psum([128, 384], "big", 2)
                    for k in range(H):
                        nc.tensor.matmul(
                            pt,
                            wv_t[k][0:D, m * 128 : (m + 1) * 128],
                            mcat[k][0:D, nh * 384 : (nh + 1) * 384],
                            start=(k == 0),
                            stop=(k == H - 1),
                        )
                    eng_copy(w2[m][:, nh * 384 : (nh + 1) * 384], pt, "da")

            # bias row = bv @ Mcat + b_proj
            bias_f = smalls.tile([1, C], F32, tag="biasf", name="biasf")
            for nh in range(2):
                pt = psum([128, 384], "mc", 2)[0:1, :]
                for k in range(H):
                    nc.tensor.matmul(
                        pt,
                        bv_col[k][0:D, :],
                        mcat[k][0:D, nh * 384 : (nh + 1) * 384],
                        start=(k == 0),
                        stop=(k == H - 1),
                    )
                nc.vector.tensor_add(
                    bias_f[:, nh * 384 : (nh + 1) * 384],
                    bp_f[:, nh * 384 : (nh + 1) * 384],
                    pt,
                )
            bias16 = smalls.tile([1, C], F16, tag="bias16", name=f"bias16_{b}")
            nc.vector.tensor_copy(bias16, bias_f)
            # replicated f32 bias (for fused bias-add in phase C copies)
            bias_rep = smalls.tile([128, C], F32, tag="brep", name=f"brep{b}")
            for nh in range(2):
                pt = psum([128, 384], "mc", 2)
                nc.tensor.matmul(
                    pt, ones_row16, bias16[:, nh * 384 : (nh + 1) * 384],
                    start=True, stop=True,
                )
                nc.vector.tensor_copy(bias_rep[:, nh * 384 : (nh + 1) * 384], pt)

            _mark(nc, "phaseC")
            # final = xT.T @ W2 + bias
            for nn in range(NT):
                ot = outpool.tile([128, C], F32, tag="ot", name="ot")
                for nh in range(2):
                    pt = psum([128, 384], "big", 2)
                    sel = (nn * 2 + nh) % 4
                    for k in range(CK):
                        nc.tensor.matmul(
                            pt,
                            xTg[k // 3][:, nn, k % 3, :],
                            w2[k][:, nh * 384 : (nh + 1) * 384],
                            start=(k == 0),
                            stop=(k == CK - 1) if sel != 2 else False,
                        )
                    dst = ot[:, nh * 384 : (nh + 1) * 384]
                    brep = bias_rep[:, nh * 384 : (nh + 1) * 384]
                    if sel == 2:
                        # Act copy; bias added via rank-1 matmul
                        nc.tensor.matmul(
                            pt, ones_row16, bias16[:, nh * 384 : (nh + 1) * 384],
                            start=False, stop=True,
                        )
                        nc.scalar.copy(dst, pt)
                    else:
                        nc.vector.tensor_add(dst, brep, pt)
                r0 = nn * 128
                nc.sync.dma_start(out[b, r0 : r0 + 128, :], ot)


_CACHED_NC = None


def _get_nc():
    global _CACHED_NC
    if _CACHED_NC is not None:
        return _CACHED_NC
    nc = bacc.Bacc("TRN2", debug=False, num_devices=NCORES)
    aps = {
        "x": nc.dram_tensor("x", (BLOC, N, C), F32, kind="ExternalInput").ap(),
        "w_qkv": nc.dram_tensor("w_qkv", (3 * C, C), F32, kind="ExternalInput").ap(),
        "b_qkv": nc.dram_tensor("b_qkv", (3 * C,), F32, kind="ExternalInput").ap(),
        "w_proj": nc.dram_tensor("w_proj", (C, C), F32, kind="ExternalInput").ap(),
        "b_proj": nc.dram_tensor("b_proj", (C,), F32, kind="ExternalInput").ap(),
        "out": nc.dram_tensor("out", (BLOC, N, C), F32, kind="ExternalOutput").ap(),
    }
    with tile.TileContext(nc) as tc:
        _build_kernel_body(nc, tc, aps)
    nc.compile()
    _CACHED_NC = nc
    return nc


def kernel(**inputs):
    x = np.ascontiguousarray(inputs["x"], dtype=np.float32)
    w_qkv = np.ascontiguousarray(inputs["w_qkv"], dtype=np.float32)
    b_qkv = np.ascontiguousarray(inputs["b_qkv"], dtype=np.float32)
    w_proj = np.ascontiguousarray(inputs["w_proj"], dtype=np.float32)
    b_proj = np.ascontiguousarray(inputs["b_proj"], dtype=np.float32)

    nc = _get_nc()
    in_maps = [
        {
            "x": x[i * BLOC : (i + 1) * BLOC],
            "w_qkv": w_qkv,
            "b_qkv": b_qkv,
            "w_proj": w_proj,
            "b_proj": b_proj,
        }
        for i in range(NCORES)
    ]
    res = bass_utils.run_bass_kernel_spmd(nc, in_maps, core_ids=list(range(NCORES)))
    return np.concatenate([res.results[i]["out"] for i in range(NCORES)], axis=0)


# revision 13
# speedup vs baseline: 1.1062x; 1.1062x over previous
"""ChannelAttentionV2 Trainium2 kernel (8 NeuronCores, data-parallel over batch).

Math (per batch b, per head h; N=4096 tokens, C=768, D=96):
  q = x @ wq.T + bq ; k = x @ wk.T + bk ; v = x @ wv.T + bv
  L = (q * N^-0.5).T @ k            [D, D] logits
  A = softmax(L, axis=-1)
  out_h = (A @ v.T).T               [N, D]
  final = concat_h(out_h) @ w_proj.T + b_proj

Kernel reformulation (per core, 2 batches), all matmuls in fp16
(psum accumulation is fp32; softmax in fp32):
  G  = x.T @ x  (upper blocks only, lower restored by PE-transpose symmetry)
  xT = PE-transpose of x tiles, kept in SBUF (feeds phase C + column sums)
  cs = free-dim reduce of xT (DVE)  ->  csT columns directly
  L  = s*(wq G wk.T + bq (x) u + sq (x) bk), u = sk + N*bk, sq/sk = cs @ wq/wk.T
  A  = softmax(L)
  Mcat_h[e,co] = sum_d A[d,e] w_proj[co, h*96+d]
  W2 = wv.T @ Mcat ; bias_row = bv @ Mcat + b_proj
  final = xT.T @ W2 + bias_row   (bias via rank-1 matmul or fused DVE/Pool add)
"""

import numpy as np

import concourse.bass as bass
import concourse.mybir as mybir
import concourse.tile as tile
from concourse import bacc
from concourse import bass_utils
from concourse.masks import make_identity

F32 = mybir.dt.float32
F16 = mybir.dt.float16

NCORES = 8
B_TOT = 16
BLOC = B_TOT // NCORES  # 2 batches per core
N = 4096
C = 768
H = 8
D = 96
CK = C // 128  # 6 chunks of channels
NT = N // 128  # 32 token tiles per batch
SCALE = float(N) ** -0.5  # 1/64

PHASE_MARKS = []  # (instruction_id_watermark, label) for profiling


def _mark(nc, label):
    try:
        name = nc.get_next_instruction_name()  # consumes one id
        PHASE_MARKS.append((int(name.split("-")[1]), label))
    except Exception:
        pass


def _build_kernel_body(nc, tc, aps):
    x = aps["x"]
    w_qkv = aps["w_qkv"]
    b_qkv = aps["b_qkv"]
    w_proj = aps["w_proj"]
    b_proj = aps["b_proj"]
    out = aps["out"]

    import contextlib

    ctx = contextlib.ExitStack()
    with ctx:
        singles = ctx.enter_context(tc.tile_pool(name="singles", bufs=1))
        xpool = ctx.enter_context(tc.tile_pool(name="xpool", bufs=8))
        wnpool = ctx.enter_context(tc.tile_pool(name="wnpool", bufs=2))
        wt_pool = ctx.enter_context(tc.tile_pool(name="wt", bufs=1))
        g_pool = ctx.enter_context(tc.tile_pool(name="gpool", bufs=1))
        a1_pool = ctx.enter_context(tc.tile_pool(name="a1", bufs=1))
        mcat_pool = ctx.enter_context(tc.tile_pool(name="mcat", bufs=1))
        w2_pool = ctx.enter_context(tc.tile_pool(name="w2", bufs=1))
        xt_pool = ctx.enter_context(tc.tile_pool(name="xt", bufs=2))
        outpool = ctx.enter_context(tc.tile_pool(name="outp", bufs=2))
        smalls = ctx.enter_context(tc.tile_pool(name="smalls", bufs=1))
        ps = ctx.enter_context(tc.tile_pool(name="ps", bufs=1, space="PSUM"))

        _psum_ctr = [0]

        def psum(shape, tag, bufs, dtype=F32):
            _psum_ctr[0] += 1
            return ps.tile(
                shape, dtype, tag=tag, bufs=bufs, name=f"ps_{tag}_{_psum_ctr[0]}"
            )

        # round-robin copy engine (gpsimd cannot access PSUM)
        _rr = [0]

        def eng_copy(dst, src, pattern="da"):
            e = pattern[_rr[0] % len(pattern)]
            _rr[0] += 1
            if e == "d":
                nc.vector.tensor_copy(dst, src)
            else:
                nc.scalar.copy(dst, src)

        # --------- tiny constants ---------
        ident16 = singles.tile([128, 128], F16)
        make_identity(nc, ident16)
        ones_row16 = singles.tile([1, 128], F16)
        nc.vector.memset(ones_row16, 1.0)
        ones_col16 = singles.tile([128, 1], F16)
        nc.vector.memset(ones_col16, 1.0)

        # persistent weight tiles (all fp16)
        wt_qk = [
            wt_pool.tile([128, 2 * C], F16, tag=f"wtqk{j}", name=f"wtqk{j}")
            for j in range(CK)
        ]
        wpT = [
            wt_pool.tile([128, C], F16, tag=f"wpT{h}", name=f"wpT{h}")
            for h in range(H)
        ]
        wv_t = [
            wt_pool.tile([128, C], F16, tag=f"wv{h}", name=f"wv{h}")
            for h in range(H)
        ]
        bq_r = singles.tile([1, C], F16)
        bk_r = singles.tile([1, C], F16)
        bk_f = singles.tile([1, C], F32)
        bp_f = singles.tile([1, C], F32)
        bv_col = [
            singles.tile([128, 1], F16, tag=f"bv{h}", name=f"bv{h}") for h in range(H)
        ]

        def emit_setup():
            # wt_qk[j][c in chunk j, 1536] = w_qkv[0:1536, :].T  (fp16)
            for i in range(2 * CK):  # 12 row-chunks of w_qkv[0:1536]
                nat = wnpool.tile([128, C], F16, tag="wn", name="wnat")
                nc.gpsimd.dma_start(nat, w_qkv[i * 128 : (i + 1) * 128, :])
                for jg in range(2):
                    pt = psum([128, 384], "tp", 2, F16)
                    for j3 in range(3):
                        j = jg * 3 + j3
                        nc.tensor.transpose(
                            pt[:, j3 * 128 : (j3 + 1) * 128],
                            nat[:, j * 128 : (j + 1) * 128],
                            ident16,
                        )
                    for j3 in range(3):
                        j = jg * 3 + j3
                        eng_copy(
                            wt_qk[j][:, i * 128 : (i + 1) * 128],
                            pt[:, j3 * 128 : (j3 + 1) * 128],
                        )

            # wpT[h][d(96), co=768] = w_proj[:, h*96+d].T  (fp16)
            for i in range(CK):  # co-chunks of w_proj
                nat = wnpool.tile([128, C], F16, tag="wn", name="wpnat")
                nc.gpsimd.dma_start(nat, w_proj[i * 128 : (i + 1) * 128, :])
                for hg in range(4):  # head groups of 2
                    pt = psum([128, 384], "tp", 2, F16)
                    for h2 in range(2):
                        h = hg * 2 + h2
                        nc.tensor.transpose(
                            pt[0:D, h2 * 128 : (h2 + 1) * 128],
                            nat[:, h * D : (h + 1) * D],
                            ident16,
                        )
                    for h2 in range(2):
                        h = hg * 2 + h2
                        eng_copy(
                            wpT[h][0:D, i * 128 : (i + 1) * 128],
                            pt[0:D, h2 * 128 : (h2 + 1) * 128],
                        )

            # wv_t[h][e(96), ci=768] = w_qkv[2C + h*96 + e, :]  (fp16)
            for h in range(H):
                nc.gpsimd.dma_start(
                    wv_t[h][0:D, :], w_qkv[2 * C + h * D : 2 * C + (h + 1) * D, :]
                )

            # bias rows
            nc.gpsimd.dma_start(bq_r, b_qkv[None, 0:C])
            nc.gpsimd.dma_start(bk_r, b_qkv[None, C : 2 * C])
            nc.sync.dma_start(bk_f, b_qkv[None, C : 2 * C])
            for h in range(H):
                nc.gpsimd.dma_start(
                    bv_col[h][0:D, :],
                    b_qkv[2 * C + h * D : 2 * C + (h + 1) * D, None],
                )
            nc.sync.dma_start(bp_f, b_proj[None, :])

        # ---------------- per batch ----------------
        for b in range(BLOC):
            _mark(nc, "phaseA")
            # x tile loads: 8 cast-DMAs of 4 token-tiles each (f32 -> fp16)
            xt4 = []
            for i in range(NT // 4):
                xt = xpool.tile([128, 4, C], F16, tag="xt", name="xt")
                r0 = i * 512
                nc.gpsimd.dma_start(
                    xt,
                    x[b, r0 : r0 + 512, :].rearrange("(t p) c -> p t c", p=128),
                )
                xt4.append(xt)

            def xts(kk):
                return xt4[kk >> 2][:, kk & 3, :]

            _mark(nc, "xT")
            # PE-transpose x -> xT (fp16), kept in SBUF for phase C
            # layout: xTg[g][c-part(128), tok-tile(32), j(3), tok(128)]
            # where channel chunk k = 3*g + j
            xTg = [
                xt_pool.tile([128, NT, 3, 128], F16, tag="xT", name=f"xT{b}_{g}")
                for g in range(2)
            ]
            for kk in range(NT):
                for g in range(2):
                    pt = psum([128, 384], "tp", 2, F16)
                    for j in range(3):
                        nc.tensor.transpose(
                            pt[:, j * 128 : (j + 1) * 128],
                            xts(kk)[:, (3 * g + j) * 128 : (3 * g + j + 1) * 128],
                            ident16,
                        )
                    eng_copy(xTg[g][:, kk, :, :], pt, "dda")

            _mark(nc, "gram")
            # G = x.T @ x: upper 384-wide blocks, 32-matmul psum accumulation
            g_t = [
                g_pool.tile([128, C], F16, tag=f"g{m}", name=f"g{m}")
                for m in range(CK)
            ]
            for m in range(CK):
                for nh in range(2):
                    if m * 128 >= (nh + 1) * 384:
                        continue  # below-diagonal half: restored by symmetry
                    pt = psum([128, 384], "big", 2)
                    for kk in range(NT):
                        nc.tensor.matmul(
                            pt,
                            xts(kk)[:, m * 128 : (m + 1) * 128],
                            xts(kk)[:, nh * 384 : (nh + 1) * 384],
                            start=(kk == 0),
                            stop=(kk == NT - 1),
                        )
                    nc.vector.tensor_copy(g_t[m][:, nh * 384 : (nh + 1) * 384], pt)

            _mark(nc, "colsum")
            # column sums on PE (rank-1 ones matmuls), then transpose to csT
            cs16 = smalls.tile([1, C], F16, tag="cs16", name="cs16")
            for nh in range(2):
                pt = psum([128, 384], "mc", 2)[0:1, :]
                for kk in range(NT):
                    nc.tensor.matmul(
                        pt,
                        ones_col16,
                        xts(kk)[:, nh * 384 : (nh + 1) * 384],
                        start=(kk == 0),
                        stop=(kk == NT - 1),
                    )
                nc.vector.tensor_copy(cs16[:, nh * 384 : (nh + 1) * 384], pt)
            csT = smalls.tile([128, CK], F16, tag="csT16", name="csT16")
            ptc = psum([128, 384], "mc", 2, F16)
            for j in range(CK):
                # psum fp16 writes must be 4-byte aligned: use even columns
                nc.tensor.transpose(
                    ptc[:, 2 * j : 2 * j + 1],
                    cs16[:, j * 128 : (j + 1) * 128],
                    ident16[0:1, 0:1],
                )
            nc.vector.tensor_copy(csT, ptc[:, 0 : 2 * CK : 2])

            if b == 0:
                _mark(nc, "setup")
                emit_setup()

            _mark(nc, "gmirror")
            # mirror below-diagonal blocks of G by transposing the upper ones
            for mi in range(3, CK):
                pt = psum([128, 384], "mc", 2, F16)
                for mj in range(3):
                    nc.tensor.transpose(
                        pt[:, mj * 128 : (mj + 1) * 128],
                        g_t[mj][:, mi * 128 : (mi + 1) * 128],
                        ident16,
                    )
                nc.vector.tensor_copy(g_t[mi][:, 0:384], pt)

            _mark(nc, "sall")
            # s = cs @ [wq|wk].T : sq (fp16 rank-1 lhsT) and sk (f32, for u)
            sq_r = smalls.tile([1, C], F16, tag="sqr", name="sqr")
            sk_f = smalls.tile([1, C], F32, tag="skf", name="skf")
            for seg in range(4):
                pt = psum([128, 384], "mc", 2)[0:1, :]
                for j in range(CK):
                    nc.tensor.matmul(
                        pt,
                        csT[:, j : j + 1],
                        wt_qk[j][:, seg * 384 : (seg + 1) * 384],
                        start=(j == 0),
                        stop=(j == CK - 1),
                    )
                if seg < 2:
                    nc.vector.tensor_copy(sq_r[:, seg * 384 : (seg + 1) * 384], pt)
                else:
                    nc.vector.tensor_copy(
                        sk_f[:, (seg - 2) * 384 : (seg - 1) * 384], pt
                    )

            # u = sk + N * bk   (fp16 row)
            u_f = smalls.tile([1, C], F32, tag="uf", name="uf")
            u_r = smalls.tile([1, C], F16, tag="ur", name="ur")
            nc.vector.tensor_scalar(u_f, bk_f, float(N), None, op0=mybir.AluOpType.mult)
            nc.vector.tensor_add(u_f, u_f, sk_f)
            nc.vector.tensor_copy(u_r, u_f)

            _mark(nc, "A1T")
            # A1T[c', d_all] = sum_c G[c, c'] * wq[d_all, c]
            a1t = [
                a1_pool.tile([128, C], F16, tag=f"a1t{m}", name=f"a1t{m}")
                for m in range(CK)
            ]
            for m in range(CK):
                for nh in range(2):
                    pt = psum([128, 384], "big", 2)
                    for k in range(CK):
                        nc.tensor.matmul(
                            pt,
                            g_t[k][:, m * 128 : (m + 1) * 128],
                            wt_qk[k][:, nh * 384 : (nh + 1) * 384],
                            start=(k == 0),
                            stop=(k == CK - 1),
                        )
                    eng_copy(a1t[m][:, nh * 384 : (nh + 1) * 384], pt, "da")

            _mark(nc, "heads")
            # per-head logits + softmax + Mcat
            mcat = [
                mcat_pool.tile([128, C], F16, tag=f"mcat{h}", name=f"mcat{h}")
                for h in range(H)
            ]
            for h in range(H):
                lp = psum([128, 96], "attn", 2)[0:D, :]
                for k in range(CK):
                    nc.tensor.matmul(
                        lp,
                        a1t[k][:, h * D : (h + 1) * D],
                        wt_qk[k][:, C + h * D : C + (h + 1) * D],
                        start=(k == 0),
                        stop=False,
                    )
                # rank-1 bias terms: bq (x) u  and  sq (x) bk
                nc.tensor.matmul(
                    lp,
                    bq_r[:, h * D : (h + 1) * D],
                    u_r[:, h * D : (h + 1) * D],
                    start=False,
                    stop=False,
                )
                nc.tensor.matmul(
                    lp,
                    sq_r[:, h * D : (h + 1) * D],
                    bk_r[:, h * D : (h + 1) * D],
                    start=False,
                    stop=True,
                )
                # softmax over free dim, scale folded into the exp
                negm = smalls.tile([128, 1], F32, tag="negm", name="negm")[0:D, :]
                nc.vector.tensor_reduce(
                    negm, lp, axis=mybir.AxisListType.X, op=mybir.AluOpType.max,
                    negate=True,
                )
                negm_s = smalls.tile([128, 1], F32, tag="negms", name="negms")[0:D, :]
                nc.vector.tensor_scalar_mul(negm_s, negm, SCALE)
                p_t = smalls.tile([128, 96], F32, tag="pt", name="pt")[0:D, :]
                ssum = smalls.tile([128, 1], F32, tag="ssum", name="ssum")[0:D, :]
                nc.scalar.activation(
                    p_t, lp, mybir.ActivationFunctionType.Exp,
                    bias=negm_s, scale=SCALE, accum_out=ssum,
                )
                rinv = smalls.tile([128, 1], F32, tag="rinv", name="rinv")[0:D, :]
                nc.vector.reciprocal(rinv, ssum)
                attn16 = smalls.tile([128, 96], F16, tag="attn16", name="attn16")[
                    0:D, :
                ]
                nc.vector.tensor_scalar_mul(attn16, p_t, rinv)
                # Mcat_h[e, co] = sum_d attn[d, e] * wpT[h][d, co]
                for nh in range(2):
                    pt = psum([128, 384], "mc", 2)[0:D, :]
                    nc.tensor.matmul(
                        pt, attn16, wpT[h][0:D, nh * 384 : (nh + 1) * 384],
                        start=True, stop=True,
                    )
                    eng_copy(mcat[h][0:D, nh * 384 : (nh + 1) * 384], pt, "da")

            _mark(nc, "W2")
            # W2 = wv.T-contract @ Mcat   [ci, co] (fp16)
            # reuses the G tiles' storage: g is dead after A1T
            w2 = [
                g_pool.tile([128, C], F16, tag=f"g{m}", name=f"w2{m}")
                for m in range(CK)
            ]
            for m in range(CK):
                for nh in range(2):
                    pt = psum([128, 384], "big", 2)
                    for k in range(H):
                        nc.tensor.matmul(
                            pt,
                            wv_t[k][0:D, m * 128 : (m + 1) * 128],
                            mcat[k][0:D, nh * 384 : (nh + 1) * 384],
                            start=(k == 0),
                            stop=(k == H - 1),
                        )
                    eng_copy(w2[m][:, nh * 384 : (nh + 1) * 384], pt, "da")

            # bias row = bv @ Mcat + b_proj
            bias_f = smalls.tile([1, C], F32, tag="biasf", name="biasf")
            for nh in range(2):
                pt = psum([128, 384], "mc", 2)[0:1, :]
                for k in range(H):
                    nc.tensor.matmul(
                        pt,
                        bv_col[k][0:D, :],
                        mcat[k][0:D, nh * 384 : (nh + 1) * 384],
                        start=(k == 0),
                        stop=(k == H - 1),
                    )
                nc.vector.tensor_add(
                    bias_f[:, nh * 384 : (nh + 1) * 384],
                    bp_f[:, nh * 384 : (nh + 1) * 384],
                    pt,
                )
            bias16 = smalls.tile([1, C], F16, tag="bias16", name=f"bias16_{b}")
            nc.vector.tensor_copy(bias16, bias_f)
            # replicated f32 bias (for fused bias-add in phase C copies)
            bias_rep = smalls.tile([128, C], F32, tag="brep", name=f"brep{b}")
            for nh in range(2):
                pt = psum([128, 384], "mc", 2)
                nc.tensor.matmul(
                    pt, ones_row16, bias16[:, nh * 384 : (nh + 1) * 384],
                    start=True, stop=True,
                )
                nc.vector.tensor_copy(bias_rep[:, nh * 384 : (nh + 1) * 384], pt)

            _mark(nc, "phaseC")
            # final = xT.T @ W2 + bias
            for nn in range(NT):
                ot = outpool.tile([128, C], F32, tag="ot", name="ot")
                for nh in range(2):
                    pt = psum([128, 384], "big", 2)
                    sel = (nn * 2 + nh) % 4
                    for k in range(CK):
                        nc.tensor.matmul(
                            pt,
                            xTg[k // 3][:, nn, k % 3, :],
                            w2[k][:, nh * 384 : (nh + 1) * 384],
                            start=(k == 0),
                            stop=(k == CK - 1) if sel != 2 else False,
                        )
                    dst = ot[:, nh * 384 : (nh + 1) * 384]
                    brep = bias_rep[:, nh * 384 : (nh + 1) * 384]
                    if sel == 2:
                        # Act copy; bias added via rank-1 matmul
                        nc.tensor.matmul(
                            pt, ones_row16, bias16[:, nh * 384 : (nh + 1) * 384],
                            start=False, stop=True,
                        )
                        nc.scalar.copy(dst, pt)
                    else:
                        nc.vector.tensor_add(dst, brep, pt)
                r0 = nn * 128
                nc.sync.dma_start(out[b, r0 : r0 + 128, :], ot)


_CACHED_NC = None


def _get_nc():
    global _CACHED_NC
    if _CACHED_NC is not None:
        return _CACHED_NC
    nc = bacc.Bacc("TRN2", debug=False, num_devices=NCORES)
    aps = {
        "x": nc.dram_tensor("x", (BLOC, N, C), F32, kind="ExternalInput").ap(),
        "w_qkv": nc.dram_tensor("w_qkv", (3 * C, C), F32, kind="ExternalInput").ap(),
        "b_qkv": nc.dram_tensor("b_qkv", (3 * C,), F32, kind="ExternalInput").ap(),
        "w_proj": nc.dram_tensor("w_proj", (C, C), F32, kind="ExternalInput").ap(),
        "b_proj": nc.dram_tensor("b_proj", (C,), F32, kind="ExternalInput").ap(),
        "out": nc.dram_tensor("out", (BLOC, N, C), F32, kind="ExternalOutput").ap(),
    }
    with tile.TileContext(nc) as tc:
        _build_kernel_body(nc, tc, aps)
    nc.compile()
    _CACHED_NC = nc
    return nc


def kernel(**inputs):
    x = np.ascontiguousarray(inputs["x"], dtype=np.float32)
    w_qkv = np.ascontiguousarray(inputs["w_qkv"], dtype=np.float32)
    b_qkv = np.ascontiguousarray(inputs["b_qkv"], dtype=np.float32)
    w_proj = np.ascontiguousarray(inputs["w_proj"], dtype=np.float32)
    b_proj = np.ascontiguousarray(inputs["b_proj"], dtype=np.float32)

    nc = _get_nc()
    in_maps = [
        {
            "x": x[i * BLOC : (i + 1) * BLOC],
            "w_qkv": w_qkv,
            "b_qkv": b_qkv,
            "w_proj": w_proj,
            "b_proj": b_proj,
        }
        for i in range(NCORES)
    ]
    res = bass_utils.run_bass_kernel_spmd(nc, in_maps, core_ids=list(range(NCORES)))
    return np.concatenate([res.results[i]["out"] for i in range(NCORES)], axis=0)


# revision 18
# speedup vs baseline: 1.1923x; 1.0778x over previous
"""ChannelAttentionV2 Trainium2 kernel (8 NeuronCores, data-parallel over batch).

Math (per batch b, per head h; N=4096 tokens, C=768, D=96):
  q = x @ wq.T + bq ; k = x @ wk.T + bk ; v = x @ wv.T + bv
  L = (q * N^-0.5).T @ k            [D, D] logits
  A = softmax(L, axis=-1)
  out_h = (A @ v.T).T               [N, D]
  final = concat_h(out_h) @ w_proj.T + b_proj

Kernel reformulation (per core, 2 batches), all matmuls in fp16
(psum accumulation is fp32; softmax in fp32):
  G  = x.T @ x  (upper blocks only, lower restored by PE-transpose symmetry)
  xT = PE-transpose of x tiles, kept in SBUF (feeds phase C + column sums)
  cs = free-dim reduce of xT (DVE)  ->  csT columns directly
  L  = s*(wq G wk.T + bq (x) u + sq (x) bk), u = sk + N*bk, sq/sk = cs @ wq/wk.T
  A  = softmax(L)
  Mcat_h[e,co] = sum_d A[d,e] w_proj[co, h*96+d]
  W2 = wv.T @ Mcat ; bias_row = bv @ Mcat + b_proj
  final = xT.T @ W2 + bias_row   (bias via rank-1 matmul or fused DVE/Pool add)
"""

import numpy as np

import concourse.bass as bass
import concourse.mybir as mybir
import concourse.tile as tile
from concourse import bacc
from concourse import bass_utils
from concourse.masks import make_identity

F32 = mybir.dt.float32
F16 = mybir.dt.float16

NCORES = 8
B_TOT = 16
BLOC = B_TOT // NCORES  # 2 batches per core
N = 4096
C = 768
H = 8
D = 96
CK = C // 128  # 6 chunks of channels
NT = N // 128  # 32 token tiles per batch
SCALE = float(N) ** -0.5  # 1/64

PHASE_MARKS = []  # (instruction_id_watermark, label) for profiling


def _mark(nc, label):
    try:
        name = nc.get_next_instruction_name()  # consumes one id
        PHASE_MARKS.append((int(name.split("-")[1]), label))
    except Exception:
        pass


def _build_kernel_body(nc, tc, aps):
    x = aps["x"]
    w_qkv = aps["w_qkv"]
    b_qkv = aps["b_qkv"]
    w_proj = aps["w_proj"]
    b_proj = aps["b_proj"]
    out = aps["out"]

    import contextlib

    ctx = contextlib.ExitStack()
    with ctx:
        singles = ctx.enter_context(tc.tile_pool(name="singles", bufs=1))
        xpool = ctx.enter_context(tc.tile_pool(name="xpool", bufs=8))
        wnpool = ctx.enter_context(tc.tile_pool(name="wnpool", bufs=2))
        wt_pool = ctx.enter_context(tc.tile_pool(name="wt", bufs=1))
        g_pool = ctx.enter_context(tc.tile_pool(name="gpool", bufs=1))
        a1_pool = ctx.enter_context(tc.tile_pool(name="a1", bufs=1))
        mcat_pool = ctx.enter_context(tc.tile_pool(name="mcat", bufs=1))
        w2_pool = ctx.enter_context(tc.tile_pool(name="w2", bufs=1))
        xt_pool = ctx.enter_context(tc.tile_pool(name="xt", bufs=2))
        outpool = ctx.enter_context(tc.tile_pool(name="outp", bufs=2))
        smalls = ctx.enter_context(tc.tile_pool(name="smalls", bufs=1))
        ps = ctx.enter_context(tc.tile_pool(name="ps", bufs=1, space="PSUM"))

        _psum_ctr = [0]

        def psum(shape, tag, bufs, dtype=F32):
            _psum_ctr[0] += 1
            return ps.tile(
                shape, dtype, tag=tag, bufs=bufs, name=f"ps_{tag}_{_psum_ctr[0]}"
            )

        # round-robin copy engine (gpsimd cannot access PSUM)
        _rr = [0]

        def eng_copy(dst, src, pattern="da"):
            e = pattern[_rr[0] % len(pattern)]
            _rr[0] += 1
            if e == "d":
                nc.vector.tensor_copy(dst, src)
            else:
                nc.scalar.copy(dst, src)

        # --------- tiny constants ---------
        ident16 = singles.tile([128, 128], F16)
        make_identity(nc, ident16)
        ones_row16 = singles.tile([1, 128], F16)
        nc.vector.memset(ones_row16, 1.0)
        ones_col16 = singles.tile([128, 1], F16)
        nc.vector.memset(ones_col16, 1.0)

        # persistent weight tiles (all fp16)
        wt_qk = [
            wt_pool.tile([128, 2 * C], F16, tag=f"wtqk{j}", name=f"wtqk{j}")
            for j in range(CK)
        ]
        wpT = [
            wt_pool.tile([128, C], F16, tag=f"wpT{h}", name=f"wpT{h}")
            for h in range(H)
        ]
        wv_t = [
            wt_pool.tile([128, C], F16, tag=f"wv{h}", name=f"wv{h}")
            for h in range(H)
        ]
        bq_r = singles.tile([1, C], F16)
        bk_r = singles.tile([1, C], F16)
        bk_f = singles.tile([1, C], F32)
        bp_f = singles.tile([1, C], F32)
        bv_col = [
            singles.tile([128, 1], F16, tag=f"bv{h}", name=f"bv{h}") for h in range(H)
        ]

        def emit_setup():
            # wt_qk[j][c in chunk j, 1536] = w_qkv[0:1536, :].T  (fp16)
            for i in range(2 * CK):  # 12 row-chunks of w_qkv[0:1536]
                nat = wnpool.tile([128, C], F16, tag="wn", name="wnat")
                nc.gpsimd.dma_start(nat, w_qkv[i * 128 : (i + 1) * 128, :])
                for jg in range(2):
                    pt = psum([128, 384], "tp", 2, F16)
                    for j3 in range(3):
                        j = jg * 3 + j3
                        nc.tensor.transpose(
                            pt[:, j3 * 128 : (j3 + 1) * 128],
                            nat[:, j * 128 : (j + 1) * 128],
                            ident16,
                        )
                    for j3 in range(3):
                        j = jg * 3 + j3
                        eng_copy(
                            wt_qk[j][:, i * 128 : (i + 1) * 128],
                            pt[:, j3 * 128 : (j3 + 1) * 128],
                        )

            # wpT[h][d(96), co=768] = w_proj[:, h*96+d].T  (fp16)
            for i in range(CK):  # co-chunks of w_proj
                nat = wnpool.tile([128, C], F16, tag="wn", name="wpnat")
                nc.gpsimd.dma_start(nat, w_proj[i * 128 : (i + 1) * 128, :])
                for hg in range(4):  # head groups of 2
                    pt = psum([128, 384], "tp", 2, F16)
                    for h2 in range(2):
                        h = hg * 2 + h2
                        nc.tensor.transpose(
                            pt[0:D, h2 * 128 : (h2 + 1) * 128],
                            nat[:, h * D : (h + 1) * D],
                            ident16,
                        )
                    for h2 in range(2):
                        h = hg * 2 + h2
                        eng_copy(
                            wpT[h][0:D, i * 128 : (i + 1) * 128],
                            pt[0:D, h2 * 128 : (h2 + 1) * 128],
                        )

            # wv_t[h][e(96), ci=768] = w_qkv[2C + h*96 + e, :]  (fp16)
            for h in range(H):
                nc.gpsimd.dma_start(
                    wv_t[h][0:D, :], w_qkv[2 * C + h * D : 2 * C + (h + 1) * D, :]
                )

            # bias rows
            nc.gpsimd.dma_start(bq_r, b_qkv[None, 0:C])
            nc.gpsimd.dma_start(bk_r, b_qkv[None, C : 2 * C])
            nc.sync.dma_start(bk_f, b_qkv[None, C : 2 * C])
            for h in range(H):
                nc.gpsimd.dma_start(
                    bv_col[h][0:D, :],
                    b_qkv[2 * C + h * D : 2 * C + (h + 1) * D, None],
                )
            nc.sync.dma_start(bp_f, b_proj[None, :])

        # ---------------- per batch ----------------
        for b in range(BLOC):
            _mark(nc, "phaseA")
            # x tile loads: 8 cast-DMAs of 4 token-tiles each (f32 -> fp16)
            xt4 = []
            for i in range(NT // 4):
                xt = xpool.tile([128, 4, C], F16, tag="xt", name="xt")
                r0 = i * 512
                nc.gpsimd.dma_start(
                    xt,
                    x[b, r0 : r0 + 512, :].rearrange("(t p) c -> p t c", p=128),
                )
                xt4.append(xt)

            def xts(kk):
                return xt4[kk >> 2][:, kk & 3, :]

            _mark(nc, "xT")
            # PE-transpose x -> xT (fp16), kept in SBUF for phase C
            # layout: xTg[g][c-part(128), tok-tile(32), j(3), tok(128)]
            # where channel chunk k = 3*g + j
            xTg = [
                xt_pool.tile([128, NT, 3, 128], F16, tag="xT", name=f"xT{b}_{g}")
                for g in range(2)
            ]
            for kk in range(NT):
                for g in range(2):
                    pt = psum([128, 384], "tp", 2, F16)
                    for j in range(3):
                        nc.tensor.transpose(
                            pt[:, j * 128 : (j + 1) * 128],
                            xts(kk)[:, (3 * g + j) * 128 : (3 * g + j + 1) * 128],
                            ident16,
                        )
                    eng_copy(xTg[g][:, kk, :, :], pt, "dda")

            _mark(nc, "gram")
            # G = x.T @ x: exact upper triangle in 128-blocks, row-grouped
            # into <=384-wide psum accumulations of 32 matmuls each
            g_t = [
                g_pool.tile([128, C], F16, tag=f"g{m}", name=f"g{m}")
                for m in range(CK)
            ]
            for i in range(CK):
                for j0 in range(i, CK, 3):
                    w = min(3, CK - j0) * 128
                    pt = psum([128, 384], "big", 2)[:, 0:w]
                    for kk in range(NT):
                        nc.tensor.matmul(
                            pt,
                            xts(kk)[:, i * 128 : (i + 1) * 128],
                            xts(kk)[:, j0 * 128 : j0 * 128 + w],
                            start=(kk == 0),
                            stop=(kk == NT - 1),
                        )
                    nc.vector.tensor_copy(
                        g_t[i][:, j0 * 128 : j0 * 128 + w], pt
                    )

            if b == 0:
                _mark(nc, "setup")
                emit_setup()

            _mark(nc, "colsum")
            # column sums on PE (rank-1 ones matmuls), then transpose to csT
            cs16 = smalls.tile([1, C], F16, tag="cs16", name="cs16")
            for nh in range(2):
                pt = psum([128, 384], "mc", 2)[0:1, :]
                for kk in range(NT):
                    nc.tensor.matmul(
                        pt,
                        ones_col16,
                        xts(kk)[:, nh * 384 : (nh + 1) * 384],
                        start=(kk == 0),
                        stop=(kk == NT - 1),
                    )
                nc.vector.tensor_copy(cs16[:, nh * 384 : (nh + 1) * 384], pt)
            csT = smalls.tile([128, CK], F16, tag="csT16", name="csT16")
            ptc = psum([128, 384], "mc", 2, F16)
            for j in range(CK):
                # psum fp16 writes must be 4-byte aligned: use even columns
                nc.tensor.transpose(
                    ptc[:, 2 * j : 2 * j + 1],
                    cs16[:, j * 128 : (j + 1) * 128],
                    ident16[0:1, 0:1],
                )
            nc.vector.tensor_copy(csT, ptc[:, 0 : 2 * CK : 2])

            _mark(nc, "gmirror")
            # mirror below-diagonal 128-blocks of G by transposing upper ones
            for mi in range(1, CK):
                for j0 in range(0, mi, 3):
                    w = min(3, mi - j0)
                    pt = psum([128, 384], "mc", 2, F16)[:, 0 : w * 128]
                    for jj in range(w):
                        nc.tensor.transpose(
                            pt[:, jj * 128 : (jj + 1) * 128],
                            g_t[j0 + jj][:, mi * 128 : (mi + 1) * 128],
                            ident16,
                        )
                    nc.vector.tensor_copy(
                        g_t[mi][:, j0 * 128 : (j0 + w) * 128], pt
                    )

            _mark(nc, "sall")
            # s = cs @ [wq|wk].T : sq (fp16 rank-1 lhsT) and sk (f32, for u)
            sq_r = smalls.tile([1, C], F16, tag="sqr", name="sqr")
            sk_f = smalls.tile([1, C], F32, tag="skf", name="skf")
            for seg in range(4):
                pt = psum([128, 384], "mc", 2)[0:1, :]
                for j in range(CK):
                    nc.tensor.matmul(
                        pt,
                        csT[:, j : j + 1],
                        wt_qk[j][:, seg * 384 : (seg + 1) * 384],
                        start=(j == 0),
                        stop=(j == CK - 1),
                    )
                if seg < 2:
                    nc.vector.tensor_copy(sq_r[:, seg * 384 : (seg + 1) * 384], pt)
                else:
                    nc.vector.tensor_copy(
                        sk_f[:, (seg - 2) * 384 : (seg - 1) * 384], pt
                    )

            # u = sk + N * bk   (fp16 row)
            u_f = smalls.tile([1, C], F32, tag="uf", name="uf")
            u_r = smalls.tile([1, C], F16, tag="ur", name="ur")
            nc.vector.tensor_scalar(u_f, bk_f, float(N), None, op0=mybir.AluOpType.mult)
            nc.vector.tensor_add(u_f, u_f, sk_f)
            nc.vector.tensor_copy(u_r, u_f)

            _mark(nc, "A1T")
            # A1T[c', d_all] = sum_c G[c, c'] * wq[d_all, c]
            a1t = [
                a1_pool.tile([128, C], F16, tag=f"a1t{m}", name=f"a1t{m}")
                for m in range(CK)
            ]
            for m in range(CK):
                for nh in range(2):
                    pt = psum([128, 384], "big" if nh == 0 else "mc", 2)
                    for k in range(CK):
                        nc.tensor.matmul(
                            pt,
                            g_t[k][:, m * 128 : (m + 1) * 128],
                            wt_qk[k][:, nh * 384 : (nh + 1) * 384],
                            start=(k == 0),
                            stop=(k == CK - 1),
                        )
                    eng_copy(a1t[m][:, nh * 384 : (nh + 1) * 384], pt, "da")

            _mark(nc, "heads")
            # per-head logits + softmax + Mcat
            mcat = [
                mcat_pool.tile([128, C], F16, tag=f"mcat{h}", name=f"mcat{h}")
                for h in range(H)
            ]
            for h in range(H):
                lp = psum([128, 96], "attn", 2)[0:D, :]
                for k in range(CK):
                    nc.tensor.matmul(
                        lp,
                        a1t[k][:, h * D : (h + 1) * D],
                        wt_qk[k][:, C + h * D : C + (h + 1) * D],
                        start=(k == 0),
                        stop=False,
                    )
                # rank-1 bias terms: bq (x) u  and  sq (x) bk
                nc.tensor.matmul(
                    lp,
                    bq_r[:, h * D : (h + 1) * D],
                    u_r[:, h * D : (h + 1) * D],
                    start=False,
                    stop=False,
                )
                nc.tensor.matmul(
                    lp,
                    sq_r[:, h * D : (h + 1) * D],
                    bk_r[:, h * D : (h + 1) * D],
                    start=False,
                    stop=True,
                )
                # softmax over free dim, scale folded into the exp
                negm = smalls.tile([128, 1], F32, tag="negm", name="negm")[0:D, :]
                nc.vector.tensor_reduce(
                    negm, lp, axis=mybir.AxisListType.X, op=mybir.AluOpType.max,
                    negate=True,
                )
                negm_s = smalls.tile([128, 1], F32, tag="negms", name="negms")[0:D, :]
                nc.vector.tensor_scalar_mul(negm_s, negm, SCALE)
                p_t = smalls.tile([128, 96], F32, tag="pt", name="pt")[0:D, :]
                ssum = smalls.tile([128, 1], F32, tag="ssum", name="ssum")[0:D, :]
                nc.scalar.activation(
                    p_t, lp, mybir.ActivationFunctionType.Exp,
                    bias=negm_s, scale=SCALE, accum_out=ssum,
                )
                rinv = smalls.tile([128, 1], F32, tag="rinv", name="rinv")[0:D, :]
                nc.vector.reciprocal(rinv, ssum)
                attn16 = smalls.tile([128, 96], F16, tag="attn16", name="attn16")[
                    0:D, :
                ]
                nc.vector.tensor_scalar_mul(attn16, p_t, rinv)
                # Mcat_h[e, co] = sum_d attn[d, e] * wpT[h][d, co]
                for nh in range(2):
                    pt = psum([128, 384], "mc", 2)[0:D, :]
                    nc.tensor.matmul(
                        pt, attn16, wpT[h][0:D, nh * 384 : (nh + 1) * 384],
                        start=True, stop=True,
                    )
                    eng_copy(mcat[h][0:D, nh * 384 : (nh + 1) * 384], pt, "da")

            _mark(nc, "W2")
            # W2 = wv.T-contract @ Mcat   [ci, co] (fp16)
            # reuses the G tiles' storage: g is dead after A1T
            w2 = [
                g_pool.tile([128, C], F16, tag=f"g{m}", name=f"w2{m}")
                for m in range(CK)
            ]
            for m in range(CK):
                for nh in range(2):
                    pt = psum([128, 384], "big" if nh == 0 else "mc", 2)
                    for k in range(H):
                        nc.tensor.matmul(
                            pt,
                            wv_t[k][0:D, m * 128 : (m + 1) * 128],
                            mcat[k][0:D, nh * 384 : (nh + 1) * 384],
                            start=(k == 0),
                            stop=(k == H - 1),
                        )
                    eng_copy(w2[m][:, nh * 384 : (nh + 1) * 384], pt, "da")

            # bias row = bv @ Mcat + b_proj
            bias_f = smalls.tile([1, C], F32, tag="biasf", name="biasf")
            for nh in range(2):
                pt = psum([128, 384], "mc", 2)[0:1, :]
                for k in range(H):
                    nc.tensor.matmul(
                        pt,
                        bv_col[k][0:D, :],
                        mcat[k][0:D, nh * 384 : (nh + 1) * 384],
                        start=(k == 0),
                        stop=(k == H - 1),
                    )
                nc.vector.tensor_add(
                    bias_f[:, nh * 384 : (nh + 1) * 384],
                    bp_f[:, nh * 384 : (nh + 1) * 384],
                    pt,
                )
            bias16 = smalls.tile([1, C], F16, tag="bias16", name=f"bias16_{b}")
            nc.vector.tensor_copy(bias16, bias_f)
            # replicated f32 bias (for fused bias-add in phase C copies)
            bias_rep = smalls.tile([128, C], F32, tag="brep", name=f"brep{b}")
            for nh in range(2):
                pt = psum([128, 384], "mc", 2)
                nc.tensor.matmul(
                    pt, ones_row16, bias16[:, nh * 384 : (nh + 1) * 384],
                    start=True, stop=True,
                )
                nc.vector.tensor_copy(bias_rep[:, nh * 384 : (nh + 1) * 384], pt)

            _mark(nc, "phaseC")
            # final = xT.T @ W2 + bias
            for nn in range(NT):
                ot = outpool.tile([128, C], F32, tag="ot", name="ot")
                for nh in range(2):
                    pt = psum([128, 384], "big" if nh == 0 else "mc", 2)
                    for k in range(CK):
                        nc.tensor.matmul(
                            pt,
                            xTg[k // 3][:, nn, k % 3, :],
                            w2[k][:, nh * 384 : (nh + 1) * 384],
                            start=(k == 0),
                            stop=(k == CK - 1),
                        )
                    # bias folded into the psum->sbuf copy
                    nc.vector.tensor_add(
                        ot[:, nh * 384 : (nh + 1) * 384],
                        bias_rep[:, nh * 384 : (nh + 1) * 384],
                        pt,
                    )
                r0 = nn * 128
                nc.sync.dma_start(out[b, r0 : r0 + 128, :], ot)


_CACHED_NC = None


def _get_nc():
    global _CACHED_NC
    if _CACHED_NC is not None:
        return _CACHED_NC
    nc = bacc.Bacc("TRN2", debug=False, num_devices=NCORES)
    aps = {
        "x": nc.dram_tensor("x", (BLOC, N, C), F32, kind="ExternalInput").ap(),
        "w_qkv": nc.dram_tensor("w_qkv", (3 * C, C), F32, kind="ExternalInput").ap(),
        "b_qkv": nc.dram_tensor("b_qkv", (3 * C,), F32, kind="ExternalInput").ap(),
        "w_proj": nc.dram_tensor("w_proj", (C, C), F32, kind="ExternalInput").ap(),
        "b_proj": nc.dram_tensor("b_proj", (C,), F32, kind="ExternalInput").ap(),
        "out": nc.dram_tensor("out", (BLOC, N, C), F32, kind="ExternalOutput").ap(),
    }
    with tile.TileContext(nc) as tc:
        _build_kernel_body(nc, tc, aps)
    nc.compile()
    _CACHED_NC = nc
    return nc


def kernel(**inputs):
    x = np.ascontiguousarray(inputs["x"], dtype=np.float32)
    w_qkv = np.ascontiguousarray(inputs["w_qkv"], dtype=np.float32)
    b_qkv = np.ascontiguousarray(inputs["b_qkv"], dtype=np.float32)
    w_proj = np.ascontiguousarray(inputs["w_proj"], dtype=np.float32)
    b_proj = np.ascontiguousarray(inputs["b_proj"], dtype=np.float32)

    nc = _get_nc()
    in_maps = [
        {
            "x": x[i * BLOC : (i + 1) * BLOC],
            "w_qkv": w_qkv,
            "b_qkv": b_qkv,
            "w_proj": w_proj,
            "b_proj": b_proj,
        }
        for i in range(NCORES)
    ]
    res = bass_utils.run_bass_kernel_spmd(nc, in_maps, core_ids=list(range(NCORES)))
    return np.concatenate([res.results[i]["out"] for i in range(NCORES)], axis=0)


# revision 22
# speedup vs baseline: 1.2663x; 1.0621x over previous
"""ChannelAttentionV2 Trainium2 kernel (8 NeuronCores, data-parallel over batch).

Math (per batch b, per head h; N=4096 tokens, C=768, D=96):
  q = x @ wq.T + bq ; k = x @ wk.T + bk ; v = x @ wv.T + bv
  L = (q * N^-0.5).T @ k            [D, D] logits
  A = softmax(L, axis=-1)
  out_h = (A @ v.T).T               [N, D]
  final = concat_h(out_h) @ w_proj.T + b_proj

Kernel reformulation (per core, 2 batches), all matmuls in fp16
(psum accumulation is fp32; softmax in fp32):
  G  = x.T @ x  (upper blocks only, lower restored by PE-transpose symmetry)
  xT = PE-transpose of x tiles, kept in SBUF (feeds phase C + column sums)
  cs = free-dim reduce of xT (DVE)  ->  csT columns directly
  L  = s*(wq G wk.T + bq (x) u + sq (x) bk), u = sk + N*bk, sq/sk = cs @ wq/wk.T
  A  = softmax(L)
  Mcat_h[e,co] = sum_d A[d,e] w_proj[co, h*96+d]
  W2 = wv.T @ Mcat ; bias_row = bv @ Mcat + b_proj
  final = xT.T @ W2 + bias_row   (bias via rank-1 matmul or fused DVE/Pool add)
"""

import numpy as np

import concourse.bass as bass
import concourse.mybir as mybir
import concourse.tile as tile
from concourse import bacc
from concourse import bass_utils
from concourse.masks import make_identity

F32 = mybir.dt.float32
F16 = mybir.dt.float16

NCORES = 8
B_TOT = 16
BLOC = B_TOT // NCORES  # 2 batches per core
N = 4096
C = 768
H = 8
D = 96
CK = C // 128  # 6 chunks of channels
NT = N // 128  # 32 token tiles per batch
SCALE = float(N) ** -0.5  # 1/64

PHASE_MARKS = []  # (instruction_id_watermark, label) for profiling


def _mark(nc, label):
    try:
        name = nc.get_next_instruction_name()  # consumes one id
        PHASE_MARKS.append((int(name.split("-")[1]), label))
    except Exception:
        pass


def _build_kernel_body(nc, tc, aps):
    x = aps["x"]
    w_qkv = aps["w_qkv"]
    b_qkv = aps["b_qkv"]
    w_proj = aps["w_proj"]
    b_proj = aps["b_proj"]
    out = aps["out"]

    import contextlib

    ctx = contextlib.ExitStack()
    with ctx:
        singles = ctx.enter_context(tc.tile_pool(name="singles", bufs=1))
        xpool = ctx.enter_context(tc.tile_pool(name="xpool", bufs=8))
        wnpool = ctx.enter_context(tc.tile_pool(name="wnpool", bufs=2))
        wt_pool = ctx.enter_context(tc.tile_pool(name="wt", bufs=1))
        g_pool = ctx.enter_context(tc.tile_pool(name="gpool", bufs=1))
        a1_pool = ctx.enter_context(tc.tile_pool(name="a1", bufs=1))
        mcat_pool = ctx.enter_context(tc.tile_pool(name="mcat", bufs=1))
        w2_pool = ctx.enter_context(tc.tile_pool(name="w2", bufs=1))
        xt_pool = ctx.enter_context(tc.tile_pool(name="xt", bufs=2))
        outpool = ctx.enter_context(tc.tile_pool(name="outp", bufs=3))
        smalls = ctx.enter_context(tc.tile_pool(name="smalls", bufs=1))
        ps = ctx.enter_context(tc.tile_pool(name="ps", bufs=1, space="PSUM"))

        _psum_ctr = [0]

        def psum(shape, tag, bufs, dtype=F32):
            _psum_ctr[0] += 1
            return ps.tile(
                shape, dtype, tag=tag, bufs=bufs, name=f"ps_{tag}_{_psum_ctr[0]}"
            )

        # round-robin copy engine (gpsimd cannot access PSUM)
        _rr = [0]

        def eng_copy(dst, src, pattern="da"):
            e = pattern[_rr[0] % len(pattern)]
            _rr[0] += 1
            if e == "d":
                nc.vector.tensor_copy(dst, src)
            else:
                nc.scalar.copy(dst, src)

        # --------- tiny constants ---------
        ident16 = singles.tile([128, 128], F16)
        make_identity(nc, ident16)
        ones_row16 = singles.tile([1, 128], F16)
        nc.vector.memset(ones_row16, 1.0)
        ones_col16 = singles.tile([128, 1], F16)
        nc.vector.memset(ones_col16, 1.0)

        # persistent weight tiles (all fp16)
        wt_qk = [
            wt_pool.tile([128, 2 * C], F16, tag=f"wtqk{j}", name=f"wtqk{j}")
            for j in range(CK)
        ]
        wpT = [
            wt_pool.tile([128, C], F16, tag=f"wpT{h}", name=f"wpT{h}")
            for h in range(H)
        ]
        wv_t = [
            wt_pool.tile([128, C], F16, tag=f"wv{h}", name=f"wv{h}")
            for h in range(H)
        ]
        bq_r = singles.tile([1, C], F16)
        bk_r = singles.tile([1, C], F16)
        bk_f = singles.tile([1, C], F32)
        bp_f = singles.tile([1, C], F32)
        bv_col = [
            singles.tile([128, 1], F16, tag=f"bv{h}", name=f"bv{h}") for h in range(H)
        ]

        def emit_setup():
            # wt_qk[j][c in chunk j, 1536] = w_qkv[0:1536, :].T  (fp16)
            for i in range(2 * CK):  # 12 row-chunks of w_qkv[0:1536]
                nat = wnpool.tile([128, C], F16, tag="wn", name="wnat")
                nc.gpsimd.dma_start(nat, w_qkv[i * 128 : (i + 1) * 128, :])
                for jg in range(2):
                    pt = psum([128, 384], "tp", 2, F16)
                    for j3 in range(3):
                        j = jg * 3 + j3
                        nc.tensor.transpose(
                            pt[:, j3 * 128 : (j3 + 1) * 128],
                            nat[:, j * 128 : (j + 1) * 128],
                            ident16,
                        )
                    for j3 in range(3):
                        j = jg * 3 + j3
                        eng_copy(
                            wt_qk[j][:, i * 128 : (i + 1) * 128],
                            pt[:, j3 * 128 : (j3 + 1) * 128],
                        )

            # wpT[h][d(96), co=768] = w_proj[:, h*96+d].T  (fp16)
            for i in range(CK):  # co-chunks of w_proj
                nat = wnpool.tile([128, C], F16, tag="wn", name="wpnat")
                nc.gpsimd.dma_start(nat, w_proj[i * 128 : (i + 1) * 128, :])
                for hg in range(4):  # head groups of 2
                    pt = psum([128, 384], "tp", 2, F16)
                    for h2 in range(2):
                        h = hg * 2 + h2
                        nc.tensor.transpose(
                            pt[0:D, h2 * 128 : (h2 + 1) * 128],
                            nat[:, h * D : (h + 1) * D],
                            ident16,
                        )
                    for h2 in range(2):
                        h = hg * 2 + h2
                        eng_copy(
                            wpT[h][0:D, i * 128 : (i + 1) * 128],
                            pt[0:D, h2 * 128 : (h2 + 1) * 128],
                        )

            # wv_t[h][e(96), ci=768] = w_qkv[2C + h*96 + e, :]  (fp16)
            for h in range(H):
                nc.gpsimd.dma_start(
                    wv_t[h][0:D, :], w_qkv[2 * C + h * D : 2 * C + (h + 1) * D, :]
                )

            # bias rows
            nc.gpsimd.dma_start(bq_r, b_qkv[None, 0:C])
            nc.gpsimd.dma_start(bk_r, b_qkv[None, C : 2 * C])
            nc.sync.dma_start(bk_f, b_qkv[None, C : 2 * C])
            for h in range(H):
                nc.gpsimd.dma_start(
                    bv_col[h][0:D, :],
                    b_qkv[2 * C + h * D : 2 * C + (h + 1) * D, None],
                )
            nc.sync.dma_start(bp_f, b_proj[None, :])

        # ---------------- per batch ----------------
        for b in range(BLOC):
            _mark(nc, "phaseA")
            # x tile loads (f32 -> fp16 cast DMAs); first load is a single
            # token-tile so PE transposes can start as early as possible
            xts_list = []
            r0 = 0
            for sz in (1, 3, 4, 4, 4, 4, 4, 4, 4):
                xt = xpool.tile(
                    [128, sz, C], F16, tag=f"xt{sz}", name="xt",
                    bufs=(7 if sz == 4 else 1),
                )
                nc.gpsimd.dma_start(
                    xt,
                    x[b, r0 : r0 + sz * 128, :].rearrange(
                        "(t p) c -> p t c", p=128
                    ),
                )
                for t in range(sz):
                    xts_list.append(xt[:, t, :])
                r0 += sz * 128

            def xts(kk):
                return xts_list[kk]

            _mark(nc, "xT")
            # PE-transpose x -> xT (fp16), kept in SBUF for phase C
            # layout: xTg[g][c-part(128), tok-tile(32), j(3), tok(128)]
            # where channel chunk k = 3*g + j
            xTg = [
                xt_pool.tile([128, NT, 3, 128], F16, tag="xT", name=f"xT{b}_{g}")
                for g in range(2)
            ]
            for kk in range(NT):
                for g in range(2):
                    pt = psum([128, 384], "tp", 2, F16)
                    for j in range(3):
                        nc.tensor.transpose(
                            pt[:, j * 128 : (j + 1) * 128],
                            xts(kk)[:, (3 * g + j) * 128 : (3 * g + j + 1) * 128],
                            ident16,
                        )
                    eng_copy(xTg[g][:, kk, :, :], pt, "dda")

            _mark(nc, "gram")
            # G = x.T @ x: exact upper triangle in 128-blocks, row-grouped
            # into <=384-wide psum accumulations of 32 matmuls each
            g_t = [
                g_pool.tile([128, C], F16, tag=f"g{m}", name=f"g{m}")
                for m in range(CK)
            ]
            for i in range(CK):
                for j0 in range(i, CK, 3):
                    w = min(3, CK - j0) * 128
                    pt = psum([128, 384], "big", 2)[:, 0:w]
                    for kk in range(NT):
                        nc.tensor.matmul(
                            pt,
                            xts(kk)[:, i * 128 : (i + 1) * 128],
                            xts(kk)[:, j0 * 128 : j0 * 128 + w],
                            start=(kk == 0),
                            stop=(kk == NT - 1),
                        )
                    nc.vector.tensor_copy(
                        g_t[i][:, j0 * 128 : j0 * 128 + w], pt
                    )

            if b == 0:
                _mark(nc, "setup")
                emit_setup()

            _mark(nc, "colsum")
            # column sums on PE (rank-1 ones matmuls), then transpose to csT
            cs16 = smalls.tile([1, C], F16, tag="cs16", name="cs16")
            for nh in range(2):
                pt = psum([128, 384], "mc", 2)[0:1, :]
                for kk in range(NT):
                    nc.tensor.matmul(
                        pt,
                        ones_col16,
                        xts(kk)[:, nh * 384 : (nh + 1) * 384],
                        start=(kk == 0),
                        stop=(kk == NT - 1),
                    )
                nc.vector.tensor_copy(cs16[:, nh * 384 : (nh + 1) * 384], pt)
            csT = smalls.tile([128, CK], F16, tag="csT16", name="csT16")
            ptc = psum([128, 384], "mc", 2, F16)
            for j in range(CK):
                # psum fp16 writes must be 4-byte aligned: use even columns
                nc.tensor.transpose(
                    ptc[:, 2 * j : 2 * j + 1],
                    cs16[:, j * 128 : (j + 1) * 128],
                    ident16[0:1, 0:1],
                )
            nc.vector.tensor_copy(csT, ptc[:, 0 : 2 * CK : 2])

            _mark(nc, "gmirror")
            # mirror below-diagonal 128-blocks of G by transposing upper ones
            for mi in range(1, CK):
                for j0 in range(0, mi, 3):
                    w = min(3, mi - j0)
                    pt = psum([128, 384], "mc", 2, F16)[:, 0 : w * 128]
                    for jj in range(w):
                        nc.tensor.transpose(
                            pt[:, jj * 128 : (jj + 1) * 128],
                            g_t[j0 + jj][:, mi * 128 : (mi + 1) * 128],
                            ident16,
                        )
                    nc.vector.tensor_copy(
                        g_t[mi][:, j0 * 128 : (j0 + w) * 128], pt
                    )

            _mark(nc, "sall")
            # s = cs @ [wq|wk].T : sq (fp16 rank-1 lhsT) and sk (f32, for u)
            sq_r = smalls.tile([1, C], F16, tag="sqr", name="sqr")
            sk_f = smalls.tile([1, C], F32, tag="skf", name="skf")
            for seg in range(4):
                pt = psum([128, 384], "mc", 2)[0:1, :]
                for j in range(CK):
                    nc.tensor.matmul(
                        pt,
                        csT[:, j : j + 1],
                        wt_qk[j][:, seg * 384 : (seg + 1) * 384],
                        start=(j == 0),
                        stop=(j == CK - 1),
                    )
                if seg < 2:
                    nc.vector.tensor_copy(sq_r[:, seg * 384 : (seg + 1) * 384], pt)
                else:
                    nc.vector.tensor_copy(
                        sk_f[:, (seg - 2) * 384 : (seg - 1) * 384], pt
                    )

            # u = sk + N * bk   (fp16 row)
            u_f = smalls.tile([1, C], F32, tag="rowf32", name="uf")
            u_r = smalls.tile([1, C], F16, tag="ur", name="ur")
            nc.vector.tensor_scalar(u_f, bk_f, float(N), None, op0=mybir.AluOpType.mult)
            nc.vector.tensor_add(u_f, u_f, sk_f)
            nc.vector.tensor_copy(u_r, u_f)

            _mark(nc, "A1T")
            # A1T[c', d_all] = sum_c G[c, c'] * wq[d_all, c]
            a1t = [
                a1_pool.tile([128, C], F16, tag=f"a1t{m}", name=f"a1t{m}")
                for m in range(CK)
            ]
            for m in range(CK):
                for nh in range(2):
                    pt = psum([128, 384], "big" if nh == 0 else "mc", 2)
                    for k in range(CK):
                        nc.tensor.matmul(
                            pt,
                            g_t[k][:, m * 128 : (m + 1) * 128],
                            wt_qk[k][:, nh * 384 : (nh + 1) * 384],
                            start=(k == 0),
                            stop=(k == CK - 1),
                        )
                    eng_copy(a1t[m][:, nh * 384 : (nh + 1) * 384], pt, "da")

            _mark(nc, "heads")
            # per-head logits + softmax + Mcat
            mcat = [
                mcat_pool.tile([128, C], F16, tag=f"mcat{h}", name=f"mcat{h}")
                for h in range(H)
            ]
            for h in range(H):
                lp = psum([128, 96], "attn", 2)[0:D, :]
                for k in range(CK):
                    nc.tensor.matmul(
                        lp,
                        a1t[k][:, h * D : (h + 1) * D],
                        wt_qk[k][:, C + h * D : C + (h + 1) * D],
                        start=(k == 0),
                        stop=False,
                    )
                # rank-1 bias terms: bq (x) u  and  sq (x) bk
                nc.tensor.matmul(
                    lp,
                    bq_r[:, h * D : (h + 1) * D],
                    u_r[:, h * D : (h + 1) * D],
                    start=False,
                    stop=False,
                )
                nc.tensor.matmul(
                    lp,
                    sq_r[:, h * D : (h + 1) * D],
                    bk_r[:, h * D : (h + 1) * D],
                    start=False,
                    stop=True,
                )
                # softmax over free dim, scale folded into the exp
                negm = smalls.tile([128, 1], F32, tag="negm", name="negm")[0:D, :]
                nc.vector.tensor_reduce(
                    negm, lp, axis=mybir.AxisListType.X, op=mybir.AluOpType.max,
                    negate=True,
                )
                negm_s = smalls.tile([128, 1], F32, tag="negms", name="negms")[0:D, :]
                nc.vector.tensor_scalar_mul(negm_s, negm, SCALE)
                p_t = smalls.tile([128, 96], F32, tag="pt", name="pt")[0:D, :]
                ssum = smalls.tile([128, 1], F32, tag="ssum", name="ssum")[0:D, :]
                nc.scalar.activation(
                    p_t, lp, mybir.ActivationFunctionType.Exp,
                    bias=negm_s, scale=SCALE, accum_out=ssum,
                )
                rinv = smalls.tile([128, 1], F32, tag="rinv", name="rinv")[0:D, :]
                nc.vector.reciprocal(rinv, ssum)
                attn16 = smalls.tile([128, 96], F16, tag="attn16", name="attn16")[
                    0:D, :
                ]
                nc.vector.tensor_scalar_mul(attn16, p_t, rinv)
                # Mcat_h[e, co] = sum_d attn[d, e] * wpT[h][d, co]
                for nh in range(2):
                    pt = psum([128, 384], "mc", 2)[0:D, :]
                    nc.tensor.matmul(
                        pt, attn16, wpT[h][0:D, nh * 384 : (nh + 1) * 384],
                        start=True, stop=True,
                    )
                    eng_copy(mcat[h][0:D, nh * 384 : (nh + 1) * 384], pt, "da")

            _mark(nc, "W2")
            # W2 = wv.T-contract @ Mcat   [ci, co] (fp16)
            # reuses the G tiles' storage: g is dead after A1T
            w2 = [
                g_pool.tile([128, C], F16, tag=f"g{m}", name=f"w2{m}")
                for m in range(CK)
            ]
            for m in range(CK):
                for nh in range(2):
                    pt = psum([128, 384], "big" if nh == 0 else "mc", 2)
                    for k in range(H):
                        nc.tensor.matmul(
                            pt,
                            wv_t[k][0:D, m * 128 : (m + 1) * 128],
                            mcat[k][0:D, nh * 384 : (nh + 1) * 384],
                            start=(k == 0),
                            stop=(k == H - 1),
                        )
                    eng_copy(w2[m][:, nh * 384 : (nh + 1) * 384], pt, "da")

            # bias row = bv @ Mcat + b_proj (storage shared with u_f)
            bias_f = smalls.tile([1, C], F32, tag="rowf32", name="biasf")
            for nh in range(2):
                pt = psum([128, 384], "mc", 2)[0:1, :]
                for k in range(H):
                    nc.tensor.matmul(
                        pt,
                        bv_col[k][0:D, :],
                        mcat[k][0:D, nh * 384 : (nh + 1) * 384],
                        start=(k == 0),
                        stop=(k == H - 1),
                    )
                nc.vector.tensor_add(
                    bias_f[:, nh * 384 : (nh + 1) * 384],
                    bp_f[:, nh * 384 : (nh + 1) * 384],
                    pt,
                )
            bias16 = smalls.tile([1, C], F16, tag="bias16", name=f"bias16_{b}")
            nc.vector.tensor_copy(bias16, bias_f)
            # replicated f32 bias (for fused bias-add in phase C copies)
            bias_rep = smalls.tile([128, C], F32, tag="brep", name=f"brep{b}")
            for nh in range(2):
                pt = psum([128, 384], "mc", 2)
                nc.tensor.matmul(
                    pt, ones_row16, bias16[:, nh * 384 : (nh + 1) * 384],
                    start=True, stop=True,
                )
                nc.vector.tensor_copy(bias_rep[:, nh * 384 : (nh + 1) * 384], pt)

            _mark(nc, "phaseC")
            # final = xT.T @ W2 + bias
            for nn in range(NT):
                ot = outpool.tile([128, C], F32, tag="ot", name="ot")
                for nh in range(2):
                    pt = psum([128, 384], "big" if nh == 0 else "mc", 2)
                    for k in range(CK):
                        nc.tensor.matmul(
                            pt,
                            xTg[k // 3][:, nn, k % 3, :],
                            w2[k][:, nh * 384 : (nh + 1) * 384],
                            start=(k == 0),
                            stop=(k == CK - 1),
                        )
                    # bias folded into the psum->sbuf copy
                    nc.vector.tensor_add(
                        ot[:, nh * 384 : (nh + 1) * 384],
                        bias_rep[:, nh * 384 : (nh + 1) * 384],
                        pt,
                    )
                r0 = nn * 128
                nc.sync.dma_start(out[b, r0 : r0 + 128, :], ot)


_CACHED_NC = None


def _get_nc():
    global _CACHED_NC
    if _CACHED_NC is not None:
        return _CACHED_NC
    nc = bacc.Bacc("TRN2", debug=False, num_devices=NCORES)
    aps = {
        "x": nc.dram_tensor("x", (BLOC, N, C), F32, kind="ExternalInput").ap(),
        "w_qkv": nc.dram_tensor("w_qkv", (3 * C, C), F32, kind="ExternalInput").ap(),
        "b_qkv": nc.dram_tensor("b_qkv", (3 * C,), F32, kind="ExternalInput").ap(),
        "w_proj": nc.dram_tensor("w_proj", (C, C), F32, kind="ExternalInput").ap(),
        "b_proj": nc.dram_tensor("b_proj", (C,), F32, kind="ExternalInput").ap(),
        "out": nc.dram_tensor("out", (BLOC, N, C), F32, kind="ExternalOutput").ap(),
    }
    with tile.TileContext(nc) as tc:
        _build_kernel_body(nc, tc, aps)
    nc.compile()
    _CACHED_NC = nc
    return nc


def kernel(**inputs):
    x = np.ascontiguousarray(inputs["x"], dtype=np.float32)
    w_qkv = np.ascontiguousarray(inputs["w_qkv"], dtype=np.float32)
    b_qkv = np.ascontiguousarray(inputs["b_qkv"], dtype=np.float32)
    w_proj = np.ascontiguousarray(inputs["w_proj"], dtype=np.float32)
    b_proj = np.ascontiguousarray(inputs["b_proj"], dtype=np.float32)

    nc = _get_nc()
    in_maps = [
        {
            "x": x[i * BLOC : (i + 1) * BLOC],
            "w_qkv": w_qkv,
            "b_qkv": b_qkv,
            "w_proj": w_proj,
            "b_proj": b_proj,
        }
        for i in range(NCORES)
    ]
    res = bass_utils.run_bass_kernel_spmd(nc, in_maps, core_ids=list(range(NCORES)))
    return np.concatenate([res.results[i]["out"] for i in range(NCORES)], axis=0)


# revision 31
# speedup vs baseline: 1.2897x; 1.0184x over previous
"""ChannelAttentionV2 Trainium2 kernel (8 NeuronCores, data-parallel over batch).

Math (per batch b, per head h; N=4096 tokens, C=768, D=96):
  q = x @ wq.T + bq ; k = x @ wk.T + bk ; v = x @ wv.T + bv
  L = (q * N^-0.5).T @ k            [D, D] logits
  A = softmax(L, axis=-1)
  out_h = (A @ v.T).T               [N, D]
  final = concat_h(out_h) @ w_proj.T + b_proj

Kernel reformulation (per core, 2 batches), all matmuls in fp16
(psum accumulation is fp32; softmax in fp32):
  G  = x.T @ x  (upper blocks only, lower restored by PE-transpose symmetry)
  xT = PE-transpose of x tiles, kept in SBUF (feeds phase C + column sums)
  cs = free-dim reduce of xT (DVE)  ->  csT columns directly
  L  = s*(wq G wk.T + bq (x) u + sq (x) bk), u = sk + N*bk, sq/sk = cs @ wq/wk.T
  A  = softmax(L)
  Mcat_h[e,co] = sum_d A[d,e] w_proj[co, h*96+d]
  W2 = wv.T @ Mcat ; bias_row = bv @ Mcat + b_proj
  final = xT.T @ W2 + bias_row   (bias via rank-1 matmul or fused DVE/Pool add)
"""

import numpy as np

import concourse.bass as bass
import concourse.mybir as mybir
import concourse.tile as tile
from concourse import bacc
from concourse import bass_utils
from concourse.masks import make_identity

F32 = mybir.dt.float32
F16 = mybir.dt.float16

NCORES = 8
B_TOT = 16
BLOC = B_TOT // NCORES  # 2 batches per core
N = 4096
C = 768
H = 8
D = 96
CK = C // 128  # 6 chunks of channels
NT = N // 128  # 32 token tiles per batch
SCALE = float(N) ** -0.5  # 1/64

PHASE_MARKS = []  # (instruction_id_watermark, label) for profiling


def _mark(nc, label):
    try:
        name = nc.get_next_instruction_name()  # consumes one id
        PHASE_MARKS.append((int(name.split("-")[1]), label))
    except Exception:
        pass


def _build_kernel_body(nc, tc, aps):
    x = aps["x"]
    w_qkv = aps["w_qkv"]
    b_qkv = aps["b_qkv"]
    w_proj = aps["w_proj"]
    b_proj = aps["b_proj"]
    out = aps["out"]

    import contextlib

    ctx = contextlib.ExitStack()
    with ctx:
        singles = ctx.enter_context(tc.tile_pool(name="singles", bufs=1))
        xpool = ctx.enter_context(tc.tile_pool(name="xpool", bufs=8))
        wnpool = ctx.enter_context(tc.tile_pool(name="wnpool", bufs=2))
        wt_pool = ctx.enter_context(tc.tile_pool(name="wt", bufs=1))
        g_pool = ctx.enter_context(tc.tile_pool(name="gpool", bufs=1))
        a1_pool = ctx.enter_context(tc.tile_pool(name="a1", bufs=1))
        mcat_pool = ctx.enter_context(tc.tile_pool(name="mcat", bufs=1))
        w2_pool = ctx.enter_context(tc.tile_pool(name="w2", bufs=1))
        xt_pool = ctx.enter_context(tc.tile_pool(name="xt", bufs=2))
        outpool = ctx.enter_context(tc.tile_pool(name="outp", bufs=3))
        smalls = ctx.enter_context(tc.tile_pool(name="smalls", bufs=1))
        ps = ctx.enter_context(tc.tile_pool(name="ps", bufs=1, space="PSUM"))

        _psum_ctr = [0]

        def psum(shape, tag, bufs, dtype=F32):
            _psum_ctr[0] += 1
            return ps.tile(
                shape, dtype, tag=tag, bufs=bufs, name=f"ps_{tag}_{_psum_ctr[0]}"
            )

        # round-robin copy engine (gpsimd cannot access PSUM)
        _rr = [0]

        def eng_copy(dst, src, pattern="da"):
            e = pattern[_rr[0] % len(pattern)]
            _rr[0] += 1
            if e == "d":
                nc.vector.tensor_copy(dst, src)
            else:
                nc.scalar.copy(dst, src)

        # --------- tiny constants ---------
        ident16 = singles.tile([128, 128], F16)
        make_identity(nc, ident16)
        ones_row16 = singles.tile([1, 128], F16)
        nc.vector.memset(ones_row16, 1.0)
        ones_col16 = singles.tile([128, 1], F16)
        nc.vector.memset(ones_col16, 1.0)

        # persistent weight tiles (all fp16)
        wt_qk = [
            wt_pool.tile([128, 2 * C], F16, tag=f"wtqk{j}", name=f"wtqk{j}")
            for j in range(CK)
        ]
        wpT = [
            wt_pool.tile([128, C], F16, tag=f"wpT{h}", name=f"wpT{h}")
            for h in range(H)
        ]
        wv_t = [
            wt_pool.tile([128, C], F16, tag=f"wv{h}", name=f"wv{h}")
            for h in range(H)
        ]
        bq_r = singles.tile([1, C], F16)
        bk_r = singles.tile([1, C], F16)
        bk_f = singles.tile([1, C], F32)
        bp_f = singles.tile([1, C], F32)
        bv_col = [
            singles.tile([128, 1], F16, tag=f"bv{h}", name=f"bv{h}") for h in range(H)
        ]

        def emit_setup():
            # wt_qk[j][c in chunk j, 1536] = w_qkv[0:1536, :].T  (fp16)
            for i in range(2 * CK):  # 12 row-chunks of w_qkv[0:1536]
                nat = wnpool.tile([128, C], F16, tag="wn", name="wnat")
                nc.gpsimd.dma_start(nat, w_qkv[i * 128 : (i + 1) * 128, :])
                for jg in range(2):
                    pt = psum([128, 384], "tp", 2, F16)
                    for j3 in range(3):
                        j = jg * 3 + j3
                        nc.tensor.transpose(
                            pt[:, j3 * 128 : (j3 + 1) * 128],
                            nat[:, j * 128 : (j + 1) * 128],
                            ident16,
                        )
                    for j3 in range(3):
                        j = jg * 3 + j3
                        eng_copy(
                            wt_qk[j][:, i * 128 : (i + 1) * 128],
                            pt[:, j3 * 128 : (j3 + 1) * 128],
                        )

            # wpT[h][d(96), co=768] = w_proj[:, h*96+d].T  (fp16)
            for i in range(CK):  # co-chunks of w_proj
                nat = wnpool.tile([128, C], F16, tag="wn", name="wpnat")
                nc.gpsimd.dma_start(nat, w_proj[i * 128 : (i + 1) * 128, :])
                for hg in range(4):  # head groups of 2
                    pt = psum([128, 384], "tp", 2, F16)
                    for h2 in range(2):
                        h = hg * 2 + h2
                        nc.tensor.transpose(
                            pt[0:D, h2 * 128 : (h2 + 1) * 128],
                            nat[:, h * D : (h + 1) * D],
                            ident16,
                        )
                    for h2 in range(2):
                        h = hg * 2 + h2
                        eng_copy(
                            wpT[h][0:D, i * 128 : (i + 1) * 128],
                            pt[0:D, h2 * 128 : (h2 + 1) * 128],
                        )

            # wv_t[h][e(96), ci=768] = w_qkv[2C + h*96 + e, :]  (fp16)
            for h in range(H):
                nc.gpsimd.dma_start(
                    wv_t[h][0:D, :], w_qkv[2 * C + h * D : 2 * C + (h + 1) * D, :]
                )

            # pre-scale the wq half (and bq below) by N^-0.5: folds the
            # softmax scale into the logits so the exp needs no extra mul
            for j in range(CK):
                nc.vector.tensor_scalar_mul(
                    wt_qk[j][:, 0:C], wt_qk[j][:, 0:C], SCALE
                )

            # bias rows
            nc.gpsimd.dma_start(bq_r, b_qkv[None, 0:C])
            nc.vector.tensor_scalar_mul(bq_r, bq_r, SCALE)
            nc.gpsimd.dma_start(bk_r, b_qkv[None, C : 2 * C])
            nc.sync.dma_start(bk_f, b_qkv[None, C : 2 * C])
            for h in range(H):
                nc.gpsimd.dma_start(
                    bv_col[h][0:D, :],
                    b_qkv[2 * C + h * D : 2 * C + (h + 1) * D, None],
                )
            nc.sync.dma_start(bp_f, b_proj[None, :])

        # ---------------- per batch ----------------
        for b in range(BLOC):
            _mark(nc, "phaseA")
            # x tile loads (f32 -> fp16 cast DMAs); first load is a single
            # token-tile so PE transposes can start as early as possible
            xts_list = []
            r0 = 0
            for sz in (1, 3, 4, 4, 4, 4, 4, 4, 4):
                xt = xpool.tile(
                    [128, sz, C], F16, tag=f"xt{sz}", name="xt",
                    bufs=(7 if sz == 4 else 1),
                )
                nc.gpsimd.dma_start(
                    xt,
                    x[b, r0 : r0 + sz * 128, :].rearrange(
                        "(t p) c -> p t c", p=128
                    ),
                )
                for t in range(sz):
                    xts_list.append(xt[:, t, :])
                r0 += sz * 128

            def xts(kk):
                return xts_list[kk]

            _mark(nc, "xT")
            # PE-transpose x -> xT (fp16), kept in SBUF for phase C
            # layout: xTg[g][c-part(128), tok-tile(32), j(3), tok(128)]
            # where channel chunk k = 3*g + j
            xTg = [
                xt_pool.tile([128, NT, 3, 128], F16, tag="xT", name=f"xT{b}_{g}")
                for g in range(2)
            ]
            for kk in range(NT):
                for g in range(2):
                    pt = psum([128, 384], "tp", 2, F16)
                    for j in range(3):
                        nc.tensor.transpose(
                            pt[:, j * 128 : (j + 1) * 128],
                            xts(kk)[:, (3 * g + j) * 128 : (3 * g + j + 1) * 128],
                            ident16,
                        )
                    eng_copy(xTg[g][:, kk, :, :], pt, "dda")

            _mark(nc, "gram")
            # G = x.T @ x: exact upper triangle in 128-blocks, row-grouped
            # into <=384-wide psum accumulations of 32 matmuls each
            g_t = [
                g_pool.tile([128, C], F16, tag=f"g{m}", name=f"g{m}")
                for m in range(CK)
            ]
            # csT rides along: out[c,1] = x_chunk.T @ ones costs ~1 cycle
            # (out free size 1) and reuses the gram matmul's lhsT
            ptc = psum([128, 96], "attn", 2)
            for i in range(CK):
                first = True
                for j0 in range(i, CK, 3):
                    w = min(3, CK - j0) * 128
                    pt = psum([128, 384], "big", 2)[:, 0:w]
                    for kk in range(NT):
                        nc.tensor.matmul(
                            pt,
                            xts(kk)[:, i * 128 : (i + 1) * 128],
                            xts(kk)[:, j0 * 128 : j0 * 128 + w],
                            start=(kk == 0),
                            stop=(kk == NT - 1),
                        )
                        if first:
                            nc.tensor.matmul(
                                ptc[:, i : i + 1],
                                xts(kk)[:, i * 128 : (i + 1) * 128],
                                ones_col16,
                                start=(kk == 0),
                                stop=(kk == NT - 1),
                            )
                    first = False
                    nc.vector.tensor_copy(
                        g_t[i][:, j0 * 128 : j0 * 128 + w], pt
                    )
            csT = smalls.tile([128, CK], F16, tag="csT16", name="csT16")
            nc.vector.tensor_copy(csT, ptc[:, 0:CK])

            if b == 0:
                _mark(nc, "setup")
                emit_setup()
            _mark(nc, "gmirror")
            # mirror below-diagonal 128-blocks of G by transposing upper ones
            for mi in range(1, CK):
                for j0 in range(0, mi, 3):
                    w = min(3, mi - j0)
                    pt = psum([128, 384], "mc", 2, F16)[:, 0 : w * 128]
                    for jj in range(w):
                        nc.tensor.transpose(
                            pt[:, jj * 128 : (jj + 1) * 128],
                            g_t[j0 + jj][:, mi * 128 : (mi + 1) * 128],
                            ident16,
                        )
                    nc.vector.tensor_copy(
                        g_t[mi][:, j0 * 128 : (j0 + w) * 128], pt
                    )

            _mark(nc, "A1T")
            # A1T[c', d_all] = sum_c G[c, c'] * wq_scaled[d_all, c]
            a1t = [
                a1_pool.tile([128, C], F16, tag=f"a1t{m}", name=f"a1t{m}")
                for m in range(CK)
            ]
            for m in range(CK):
                for nh in range(2):
                    pt = psum([128, 384], "big" if nh == 0 else "mc", 2)
                    for k in range(CK):
                        nc.tensor.matmul(
                            pt,
                            g_t[k][:, m * 128 : (m + 1) * 128],
                            wt_qk[k][:, nh * 384 : (nh + 1) * 384],
                            start=(k == 0),
                            stop=(k == CK - 1),
                        )
                    eng_copy(a1t[m][:, nh * 384 : (nh + 1) * 384], pt, "da")

            _mark(nc, "sall")
            # s = cs @ [wq|wk].T : sq (fp16 rank-1 lhsT) and sk (f32, for u)
            sq_r = smalls.tile([1, C], F16, tag="sqr", name="sqr")
            sk_f = smalls.tile([1, C], F32, tag="skf", name="skf")
            for seg in range(4):
                pt = psum([128, 384], "mc", 2)[0:1, :]
                for j in range(CK):
                    nc.tensor.matmul(
                        pt,
                        csT[:, j : j + 1],
                        wt_qk[j][:, seg * 384 : (seg + 1) * 384],
                        start=(j == 0),
                        stop=(j == CK - 1),
                    )
                if seg < 2:
                    nc.vector.tensor_copy(sq_r[:, seg * 384 : (seg + 1) * 384], pt)
                else:
                    nc.vector.tensor_copy(
                        sk_f[:, (seg - 2) * 384 : (seg - 1) * 384], pt
                    )

            # u = sk + N * bk   (fp16 row)
            u_f = smalls.tile([1, C], F32, tag="rowf32", name="uf")
            u_r = smalls.tile([1, C], F16, tag="ur", name="ur")
            nc.vector.tensor_scalar(u_f, bk_f, float(N), None, op0=mybir.AluOpType.mult)
            nc.vector.tensor_add(u_f, u_f, sk_f)
            nc.vector.tensor_copy(u_r, u_f)

            _mark(nc, "heads")
            # per-head logits + softmax + Mcat
            mcat = [
                mcat_pool.tile([128, C], F16, tag=f"mcat{h}", name=f"mcat{h}")
                for h in range(H)
            ]
            for h in range(H):
                lp = psum([128, 96], "attn", 2)[0:D, :]
                for k in range(CK):
                    nc.tensor.matmul(
                        lp,
                        a1t[k][:, h * D : (h + 1) * D],
                        wt_qk[k][:, C + h * D : C + (h + 1) * D],
                        start=(k == 0),
                        stop=False,
                    )
                # rank-1 bias terms: bq (x) u  and  sq (x) bk
                nc.tensor.matmul(
                    lp,
                    bq_r[:, h * D : (h + 1) * D],
                    u_r[:, h * D : (h + 1) * D],
                    start=False,
                    stop=False,
                )
                nc.tensor.matmul(
                    lp,
                    sq_r[:, h * D : (h + 1) * D],
                    bk_r[:, h * D : (h + 1) * D],
                    start=False,
                    stop=True,
                )
                # softmax over free dim (logits pre-scaled via wq/bq/sq)
                negm = smalls.tile([128, 1], F32, tag="negm", name="negm")[0:D, :]
                nc.vector.tensor_reduce(
                    negm, lp, axis=mybir.AxisListType.X, op=mybir.AluOpType.max,
                    negate=True,
                )
                p_t = smalls.tile([128, 96], F32, tag="pt", name="pt")[0:D, :]
                ssum = smalls.tile([128, 1], F32, tag="ssum", name="ssum")[0:D, :]
                nc.scalar.activation(
                    p_t, lp, mybir.ActivationFunctionType.Exp,
                    bias=negm, scale=1.0, accum_out=ssum,
                )
                rinv = smalls.tile([128, 1], F32, tag="rinv", name="rinv")[0:D, :]
                nc.vector.reciprocal(rinv, ssum)
                attn16 = smalls.tile([128, 96], F16, tag="attn16", name="attn16")[
                    0:D, :
                ]
                nc.vector.tensor_scalar_mul(attn16, p_t, rinv)
                # Mcat_h[e, co] = sum_d attn[d, e] * wpT[h][d, co]
                for nh in range(2):
                    pt = psum([128, 384], "mc", 2)[0:D, :]
                    nc.tensor.matmul(
                        pt, attn16, wpT[h][0:D, nh * 384 : (nh + 1) * 384],
                        start=True, stop=True,
                    )
                    eng_copy(mcat[h][0:D, nh * 384 : (nh + 1) * 384], pt, "da")

            _mark(nc, "W2")
            # W2 = wv.T-contract @ Mcat   [ci, co] (fp16)
            # reuses the G tiles' storage: g is dead after A1T
            w2 = [
                g_pool.tile([128, C], F16, tag=f"g{m}", name=f"w2{m}")
                for m in range(CK)
            ]
            for m in range(CK):
                for nh in range(2):
                    pt = psum([128, 384], "big" if nh == 0 else "mc", 2)
                    for k in range(H):
                        nc.tensor.matmul(
                            pt,
                            wv_t[k][0:D, m * 128 : (m + 1) * 128],
                            mcat[k][0:D, nh * 384 : (nh + 1) * 384],
                            start=(k == 0),
                            stop=(k == H - 1),
                        )
                    eng_copy(w2[m][:, nh * 384 : (nh + 1) * 384], pt, "da")

            # bias row = bv @ Mcat + b_proj (storage shared with u_f)
            bias_f = smalls.tile([1, C], F32, tag="rowf32", name="biasf")
            for nh in range(2):
                pt = psum([128, 384], "mc", 2)[0:1, :]
                for k in range(H):
                    nc.tensor.matmul(
                        pt,
                        bv_col[k][0:D, :],
                        mcat[k][0:D, nh * 384 : (nh + 1) * 384],
                        start=(k == 0),
                        stop=(k == H - 1),
                    )
                nc.vector.tensor_add(
                    bias_f[:, nh * 384 : (nh + 1) * 384],
                    bp_f[:, nh * 384 : (nh + 1) * 384],
                    pt,
                )
            bias16 = smalls.tile([1, C], F16, tag="bias16", name=f"bias16_{b}")
            nc.vector.tensor_copy(bias16, bias_f)
            # replicated f32 bias (for fused bias-add in phase C copies)
            bias_rep = smalls.tile([128, C], F32, tag="brep", name=f"brep{b}")
            for nh in range(2):
                pt = psum([128, 384], "mc", 2)
                nc.tensor.matmul(
                    pt, ones_row16, bias16[:, nh * 384 : (nh + 1) * 384],
                    start=True, stop=True,
                )
                nc.vector.tensor_copy(bias_rep[:, nh * 384 : (nh + 1) * 384], pt)

            _mark(nc, "phaseC")
            # final = xT.T @ W2 + bias
            for nn in range(NT):
                ot = outpool.tile([128, C], F32, tag="ot", name="ot")
                for nh in range(2):
                    pt = psum([128, 384], "big" if nh == 0 else "mc", 2)
                    for k in range(CK):
                        nc.tensor.matmul(
                            pt,
                            xTg[k // 3][:, nn, k % 3, :],
                            w2[k][:, nh * 384 : (nh + 1) * 384],
                            start=(k == 0),
                            stop=(k == CK - 1),
                        )
                    # bias folded into the psum->sbuf copy
                    nc.vector.tensor_add(
                        ot[:, nh * 384 : (nh + 1) * 384],
                        bias_rep[:, nh * 384 : (nh + 1) * 384],
                        pt,
                    )
                r0 = nn * 128
                nc.sync.dma_start(out[b, r0 : r0 + 128, :], ot)


_CACHED_NC = None


def _get_nc():
    global _CACHED_NC
    if _CACHED_NC is not None:
        return _CACHED_NC
    nc = bacc.Bacc("TRN2", debug=False, num_devices=NCORES)
    aps = {
        "x": nc.dram_tensor("x", (BLOC, N, C), F32, kind="ExternalInput").ap(),
        "w_qkv": nc.dram_tensor("w_qkv", (3 * C, C), F32, kind="ExternalInput").ap(),
        "b_qkv": nc.dram_tensor("b_qkv", (3 * C,), F32, kind="ExternalInput").ap(),
        "w_proj": nc.dram_tensor("w_proj", (C, C), F32, kind="ExternalInput").ap(),
        "b_proj": nc.dram_tensor("b_proj", (C,), F32, kind="ExternalInput").ap(),
        "out": nc.dram_tensor("out", (BLOC, N, C), F32, kind="ExternalOutput").ap(),
    }
    with tile.TileContext(nc) as tc:
        _build_kernel_body(nc, tc, aps)
    nc.compile()
    _CACHED_NC = nc
    return nc


def kernel(**inputs):
    x = np.ascontiguousarray(inputs["x"], dtype=np.float32)
    w_qkv = np.ascontiguousarray(inputs["w_qkv"], dtype=np.float32)
    b_qkv = np.ascontiguousarray(inputs["b_qkv"], dtype=np.float32)
    w_proj = np.ascontiguousarray(inputs["w_proj"], dtype=np.float32)
    b_proj = np.ascontiguousarray(inputs["b_proj"], dtype=np.float32)

    nc = _get_nc()
    in_maps = [
        {
            "x": x[i * BLOC : (i + 1) * BLOC],
            "w_qkv": w_qkv,
            "b_qkv": b_qkv,
            "w_proj": w_proj,
            "b_proj": b_proj,
        }
        for i in range(NCORES)
    ]
    res = bass_utils.run_bass_kernel_spmd(nc, in_maps, core_ids=list(range(NCORES)))
    return np.concatenate([res.results[i]["out"] for i in range(NCORES)], axis=0)
